# revision 1
# baseline (speedup 1.0000x reference)
"""Trainium2 Bass kernel for nn_SanctionImpactGNN.

Temporal GNN: per timestep t (T=8) a 2-layer GCN over a 20000-node /
320000-edge graph; node-0 ("india") embeddings over time feed a tiny GRU +
sigmoid heads -> [8] output.

Strategy
--------
Data-parallel over the T=8 graph snapshots: one snapshot per NeuronCore.

Per core (phase 1):
  * Host-side layout prep (index manipulation only): nodes are permuted by
    in-edge count (node 0 pinned to position 0) and packed into 128-node
    groups; each node owns a contiguous run of "slots" in the free dim of
    its partition row.  Every edge (dst-partitioned) is assigned one slot.
  * g1 = dis * (x @ W1) is computed on-chip (TensorE) and spilled to a DRAM
    scratch in node-position order.
  * Messages are fetched with the SWDGE dma_gather primitive (256B/edge),
    landing exactly at their slot; a DVE multiply by edge weight (slot
    layout) and a free-dim segmented reduce produce the GCN aggregation
    without any scatter.
  * Weighted degree comes free: reduce of the slot-layout edge weights.
  * Layer 2 only needs the aggregation for node 0's group (the output is
    h2[node0]), so its gather is tiny; but g2 must be produced for all
    nodes (message sources).

Phase 2 (single core): the 8 gathered india embeddings run through the GRU
and heads with tiny matmuls (bias folded in via an augmented-ones row).

All floating-point math happens on-device in fp32; the host only permutes /
packs data and indices.
"""

import numpy as np

import concourse.bacc as bacc
import concourse.mybir as mybir
import concourse.tile as tile
from concourse import bass_utils

F32 = mybir.dt.float32
I16 = mybir.dt.int16
AF = mybir.ActivationFunctionType
OP = mybir.AluOpType
AX = mybir.AxisListType

# Problem constants (hardcoded per contest contract).
T, N, E, F, H = 8, 20000, 320000, 128, 64
P = 128
INDIA = 0


class Plan:
    """Compile-time structure shared by all graphs/cores."""

    def __init__(self, n, e, f, h, W_sched, ch_s=96, slab=16):
        self.n, self.e, self.f, self.h = n, e, f, h
        self.nt = (n + P - 1) // P
        self.np_ = self.nt * P
        self.W = np.asarray(W_sched, np.int64)
        assert len(self.W) == self.nt
        self.base = np.zeros(self.nt + 1, np.int64)
        self.base[1:] = np.cumsum(self.W)
        self.S = int(self.base[-1])
        # S must give an idx tile with whole int16 columns; S*128 % 16 == 0 always.
        self.ch_s = ch_s
        self.slab = slab
        chunks = []
        g0 = 0
        while g0 < self.nt:
            g1 = g0 + 1
            while g1 < self.nt and self.base[g1 + 1] - self.base[g0] <= ch_s:
                g1 += 1
            assert self.base[g1] - self.base[g0] <= ch_s, "group wider than chunk"
            chunks.append((g0, g1))
            g0 = g1
        self.chunks = chunks

    def key(self):
        return (self.n, self.e, self.f, self.h, self.ch_s, self.slab,
                tuple(int(w) for w in self.W))


def make_plan(dst_list, n, e, f, h, **kw):
    """W_sched = per-group max in-edge count across all graphs (sorted order)."""
    nt = (n + P - 1) // P
    np_ = nt * P
    W = np.zeros(nt, np.int64)
    for dst in dst_list:
        order, cnt = _node_order(dst, n)
        cs = np.zeros(np_, np.int64)
        cs[:n] = cnt[order]
        W = np.maximum(W, cs.reshape(nt, P).max(1))
    W = np.maximum(W, 1)
    return Plan(n, e, f, h, W, **kw)


def _node_order(dst, n):
    cnt = np.bincount(dst, minlength=n).astype(np.int64)
    rest = np.argsort(-cnt, kind="stable")
    rest = rest[rest != INDIA]
    return np.concatenate([[INDIA], rest]), cnt


def graph_inputs(plan, x_t, ei_t, ew_t):
    """Per-graph, per-core input arrays (host: permutation/packing only)."""
    n, f, h = plan.n, plan.f, plan.h
    src, dst = np.asarray(ei_t[0]), np.asarray(ei_t[1])
    ne = src.shape[0]
    order, cnt = _node_order(dst, n)
    pos = np.empty(n, np.int64)
    pos[order] = np.arange(n)
    r = pos[dst]
    sidx = np.argsort(r, kind="stable")
    r_s = r[sidx]
    first = np.searchsorted(r_s, r_s, side="left")
    k = np.arange(ne) - first
    g = r_s // P
    p = r_s % P
    col = plan.base[g] + k
    assert (k < plan.W[g]).all(), "edge slot overflow vs W_sched"
    gidx = np.zeros(plan.S * P, np.int16)
    gidx[col * P + p] = pos[src[sidx]].astype(np.int16)
    ewsl = np.zeros((P, plan.S), np.float32)
    ewsl[p, col] = np.asarray(ew_t, np.float32)[sidx]
    gidx16 = np.ascontiguousarray(
        np.tile(gidx.reshape(plan.S * 8, 16).T, (8, 1)))
    xT = np.zeros((f, plan.np_), np.float32)
    xT[:, :n] = np.asarray(x_t, np.float32)[order].T
    return {"xT": xT, "ews": ewsl, "gidx": gidx16}


def build_phase1(nc, plan, stage=4):
    f32 = F32
    nt, S, f, h = plan.nt, plan.S, plan.f, plan.h
    base, W = plan.base, plan.W

    xT_d = nc.dram_tensor("xT", [P, plan.np_], f32, kind="ExternalInput")
    ew_d = nc.dram_tensor("ews", [P, S], f32, kind="ExternalInput")
    gi_d = nc.dram_tensor("gidx", [P, S * 8], I16, kind="ExternalInput")
    W1_d = nc.dram_tensor("W1", [f, h], f32, kind="ExternalInput")
    W2_d = nc.dram_tensor("W2", [h, h], f32, kind="ExternalInput")
    b1_d = nc.dram_tensor("b1B", [P, h], f32, kind="ExternalInput")
    b2_d = nc.dram_tensor("b2B", [P, h], f32, kind="ExternalInput")
    id_d = nc.dram_tensor("ident", [P, P], f32, kind="ExternalInput")
    india_d = nc.dram_tensor("india", [1, h], f32, kind="ExternalOutput")
    dbg_d = None
    if stage < 4 or stage in (20, 21, 22):
        dbg_d = nc.dram_tensor("dbg", [P, nt * h], f32, kind="ExternalOutput")

    with tile.TileContext(nc) as tc:
        with (
            tc.tile_pool(name="dram", bufs=1, space="DRAM") as dpool,
            tc.tile_pool(name="const", bufs=1) as const,
            tc.tile_pool(name="xs", bufs=2) as xpool,
            tc.tile_pool(name="gsl", bufs=2) as gslpool,
            tc.tile_pool(name="v", bufs=2) as vpool,
            tc.tile_pool(name="git", bufs=3) as gipool,
            tc.tile_pool(name="sm", bufs=6) as sm,
            tc.tile_pool(name="h1t", bufs=2) as h1tpool,
            tc.tile_pool(name="ps", bufs=4, space="PSUM") as pspool,
            tc.tile_pool(name="pst", bufs=2, space="PSUM") as pstpool,
        ):
            g1_d = dpool.tile([plan.np_, h], f32, tag="g1d")
            g2_d = dpool.tile([plan.np_, h], f32, tag="g2d")

            w1_s = const.tile([f, h], f32, tag="w1")
            w2_s = const.tile([h, h], f32, tag="w2")
            b1_s = const.tile([P, h], f32, tag="b1")
            b2_s = const.tile([P, h], f32, tag="b2")
            id_s = const.tile([P, P], f32, tag="id")
            ew_s = const.tile([P, S], f32, tag="ew")
            deg_s = const.tile([P, nt], f32, tag="deg")
            dis_s = const.tile([P, nt], f32, tag="dis")
            gb_s = const.tile([P, nt * h], f32, tag="gb")
            h1_s = const.tile([P, nt * h], f32, tag="h1")
            gb2_s = const.tile([P, h], f32, tag="gb2")

            nc.sync.dma_start(w1_s[:], W1_d[:])
            nc.sync.dma_start(w2_s[:], W2_d[:])
            nc.sync.dma_start(b1_s[:], b1_d[:])
            nc.sync.dma_start(b2_s[:], b2_d[:])
            nc.sync.dma_start(id_s[:], id_d[:])
            nc.sync.dma_start(ew_s[:], ew_d[:])

            # weighted degree + dis = rsqrt(1 + deg)
            for i in range(nt):
                nc.vector.reduce_sum(deg_s[:, i:i + 1],
                                     ew_s[:, base[i]:base[i + 1]], axis=AX.X)
            nc.scalar.activation(deg_s[:], deg_s[:], AF.Sqrt, bias=1.0)
            nc.vector.reciprocal(dis_s[:], deg_s[:])

            if stage == 0:
                nc.sync.dma_start(dbg_d[:, 0:nt], dis_s[:])
            # ---- layer 1 linear: g1 = dis * (x @ W1); gb = dis*g1 + b1
            slab = plan.slab
            for s0 in range(0, nt, slab) if (stage >= 1) else []:
                s1 = min(s0 + slab, nt)
                nk = s1 - s0
                xs = xpool.tile([P, slab * P], f32, tag="xs")
                nc.sync.dma_start(xs[:, 0:nk * P], xT_d[:, s0 * P:s1 * P])
                g1s = gslpool.tile([P, slab * h], f32, tag="gsl")
                for kk in range(nk):
                    i = s0 + kk
                    ps = pspool.tile([P, h], f32, tag="ps")
                    nc.tensor.matmul(ps[:], xs[:, kk * P:(kk + 1) * P], w1_s[:],
                                     start=True, stop=True)
                    nc.vector.tensor_scalar_mul(g1s[:, kk * h:(kk + 1) * h],
                                                ps[:], dis_s[:, i:i + 1])
                    nc.vector.tensor_scalar_mul(gb_s[:, i * h:(i + 1) * h],
                                                g1s[:, kk * h:(kk + 1) * h],
                                                dis_s[:, i:i + 1])
                    nc.vector.tensor_add(gb_s[:, i * h:(i + 1) * h],
                                         gb_s[:, i * h:(i + 1) * h], b1_s[:])
                out_ap = g1_d[s0 * P:s1 * P, :].rearrange(
                    "(k p) f -> p k f", p=P)
                nc.sync.dma_start(out_ap,
                                  g1s[:, 0:nk * h].rearrange(
                                      "p (k f) -> p k f", f=h))

            if stage == 1:
                nc.sync.dma_start(dbg_d[:], gb_s[:])
            dbg_d_stage_tile = None
            if stage in (20, 21, 22):
                dbg_d_stage_tile = const.tile([P, nt * h], f32, tag="dbgt")
            # ---- layer 1 aggregation
            # sub-stage codes: 20=gather only, 21=+multiply, 22=+reduce
            sub = stage if stage in (20, 21, 22) else None
            for (c0, c1) in plan.chunks if (stage >= 2 or sub) else []:
                b0 = int(base[c0])
                sc = int(base[c1] - b0)
                v = vpool.tile([P, plan.ch_s, h], f32, tag="v")
                git = gipool.tile([P, plan.ch_s * 8], I16, tag="git")
                nc.sync.dma_start(git[:, 0:sc * 8], gi_d[:, b0 * 8:(b0 + sc) * 8])
                nc.gpsimd.dma_gather(v[:, 0:sc, :], g1_d[:], git[:, 0:sc * 8],
                                     sc * P, sc * P, h, single_packet=False)
                if sub == 20:
                    if c0 == 0:
                        nn = min(sc * h, nt * h)
                        nc.vector.tensor_copy(
                            dbg_d_stage_tile[:, 0:nn],
                            v[:, 0:sc, :].rearrange("p a b -> p (a b)")[:, 0:nn])
                    continue
                ewb = ew_s[:, b0:b0 + sc].unsqueeze(2).broadcast_to((P, sc, h))
                nc.vector.tensor_tensor(v[:, 0:sc, :], v[:, 0:sc, :], ewb,
                                        op=OP.mult)
                if sub == 21:
                    if c0 == 0:
                        nn = min(sc * h, nt * h)
                        nc.vector.tensor_copy(
                            dbg_d_stage_tile[:, 0:nn],
                            v[:, 0:sc, :].rearrange("p a b -> p (a b)")[:, 0:nn])
                    continue
                for gi_ in range(c0, c1):
                    off = int(base[gi_] - b0)
                    wg = int(W[gi_])
                    acc = sm.tile([P, h], f32, tag="acc")
                    nc.vector.reduce_sum(
                        acc[:], v[:, off:off + wg, :].transpose([0, 2, 1]),
                        axis=AX.X)
                    t2 = sm.tile([P, h], f32, tag="t2")
                    nc.vector.tensor_scalar_mul(t2[:], acc[:],
                                                dis_s[:, gi_:gi_ + 1])
                    nc.vector.tensor_add(t2[:], t2[:],
                                         gb_s[:, gi_ * h:(gi_ + 1) * h])
                    nc.scalar.activation(h1_s[:, gi_ * h:(gi_ + 1) * h],
                                         t2[:], AF.Relu)

            if stage == 2:
                nc.sync.dma_start(dbg_d[:], h1_s[:])
            if stage in (20, 21):
                nc.sync.dma_start(dbg_d[:], dbg_d_stage_tile[:])
            if stage == 22:
                nc.sync.dma_start(dbg_d[:], h1_s[:])
            # ---- layer 2 linear: g2 = dis * (h1 @ W2)
            for s0 in range(0, nt, slab) if (stage >= 3 and stage < 20) else []:
                s1 = min(s0 + slab, nt)
                nk = s1 - s0
                g2s = gslpool.tile([P, slab * h], f32, tag="gsl")
                for kk in range(nk):
                    i = s0 + kk
                    pst = pstpool.tile([h, P], f32, tag="pst")
                    nc.tensor.transpose(pst[:], h1_s[:, i * h:(i + 1) * h],
                                        id_s[:])
                    h1t = h1tpool.tile([h, P], f32, tag="h1t")
                    nc.scalar.activation(h1t[:], pst[:], AF.Copy)
                    ps2 = pspool.tile([P, h], f32, tag="ps")
                    nc.tensor.matmul(ps2[:], h1t[:], w2_s[:],
                                     start=True, stop=True)
                    nc.vector.tensor_scalar_mul(g2s[:, kk * h:(kk + 1) * h],
                                                ps2[:], dis_s[:, i:i + 1])
                    if i == 0:
                        nc.vector.tensor_scalar_mul(gb2_s[:],
                                                    g2s[:, 0:h],
                                                    dis_s[:, 0:1])
                        nc.vector.tensor_add(gb2_s[:], gb2_s[:], b2_s[:])
                out_ap = g2_d[s0 * P:s1 * P, :].rearrange(
                    "(k p) f -> p k f", p=P)
                nc.sync.dma_start(out_ap,
                                  g2s[:, 0:nk * h].rearrange(
                                      "p (k f) -> p k f", f=h))

            if stage == 3:
                nc.sync.dma_start(dbg_d[:, 0:h], gb2_s[:])
            if stage != 4:
                nc.vector.memset(gb2_s[:, 0:1], 0.0)
                nc.sync.dma_start(india_d[:], gb2_s[0:1, 0:h])
            # ---- layer 2 aggregation, group 0 only; output = node-0 row
            w0 = int(W[0]) if stage == 4 else 0
            if stage == 4:
                v0 = vpool.tile([P, plan.ch_s, h], f32, tag="v")
                git0 = gipool.tile([P, plan.ch_s * 8], I16, tag="git")
                nc.sync.dma_start(git0[:, 0:w0 * 8], gi_d[:, 0:w0 * 8])
                nc.gpsimd.dma_gather(v0[:, 0:w0, :], g2_d[:], git0[:, 0:w0 * 8],
                                     w0 * P, w0 * P, h, single_packet=False)
                ewb0 = ew_s[:, 0:w0].unsqueeze(2).broadcast_to((P, w0, h))
                nc.vector.tensor_tensor(v0[:, 0:w0, :], v0[:, 0:w0, :], ewb0,
                                        op=OP.mult)
                acc0 = sm.tile([P, h], f32, tag="acc")
                nc.vector.reduce_sum(acc0[:],
                                     v0[:, 0:w0, :].transpose([0, 2, 1]), axis=AX.X)
                h2 = sm.tile([P, h], f32, tag="t2")
                nc.vector.tensor_scalar_mul(h2[:], acc0[:], dis_s[:, 0:1])
                nc.vector.tensor_add(h2[:], h2[:], gb2_s[:])
                out_t = sm.tile([P, h], f32, tag="outt")
                nc.scalar.activation(out_t[:], h2[:], AF.Relu)
                nc.sync.dma_start(india_d[:], out_t[0:1, :])
    nc.compile()
    return nc


def build_phase2(nc, t_steps, h):
    f32 = F32
    seq_d = nc.dram_tensor("seqT", [h, t_steps], f32, kind="ExternalInput")
    wih_d = nc.dram_tensor("WihTa", [h + 1, 3 * h], f32, kind="ExternalInput")
    whh_d = nc.dram_tensor("WhhTa", [h + 1, 3 * h], f32, kind="ExternalInput")
    hw_d = nc.dram_tensor("headWTa", [h + 1, 8], f32, kind="ExternalInput")
    out_d = nc.dram_tensor("out", [8, 1], f32, kind="ExternalOutput")

    with tile.TileContext(nc) as tc:
        with (
            tc.tile_pool(name="const", bufs=1) as const,
            tc.tile_pool(name="sm", bufs=4) as sm,
            tc.tile_pool(name="ps", bufs=1, space="PSUM") as pspool,
        ):
            wih_s = const.tile([h + 1, 3 * h], f32, tag="wih")
            whh_s = const.tile([h + 1, 3 * h], f32, tag="whh")
            hw_s = const.tile([h + 1, 8], f32, tag="hw")
            xaug = const.tile([h + 1, t_steps], f32, tag="xaug")
            haug = const.tile([h + 1, 1], f32, tag="haug")

            nc.sync.dma_start(wih_s[:], wih_d[:])
            nc.sync.dma_start(whh_s[:], whh_d[:])
            nc.sync.dma_start(hw_s[:], hw_d[:])
            nc.sync.dma_start(xaug[0:h, :], seq_d[:])
            nc.vector.memset(xaug[h:h + 1, :], 1.0)
            nc.vector.memset(haug[0:h, :], 0.0)
            nc.vector.memset(haug[h:h + 1, :], 1.0)

            for t in range(t_steps):
                xt = xaug[:, t:t + 1]
                ps_r = pspool.tile([h, 1], f32, tag="psr")
                nc.tensor.matmul(ps_r[:], wih_s[:, 0:h], xt, start=True,
                                 stop=False)
                nc.tensor.matmul(ps_r[:], whh_s[:, 0:h], haug[:], start=False,
                                 stop=True)
                ps_z = pspool.tile([h, 1], f32, tag="psz")
                nc.tensor.matmul(ps_z[:], wih_s[:, h:2 * h], xt, start=True,
                                 stop=False)
                nc.tensor.matmul(ps_z[:], whh_s[:, h:2 * h], haug[:],
                                 start=False, stop=True)
                ps_in = pspool.tile([h, 1], f32, tag="psi")
                nc.tensor.matmul(ps_in[:], wih_s[:, 2 * h:3 * h], xt,
                                 start=True, stop=True)
                ps_hn = pspool.tile([h, 1], f32, tag="psh")
                nc.tensor.matmul(ps_hn[:], whh_s[:, 2 * h:3 * h], haug[:],
                                 start=True, stop=True)
                r = sm.tile([h, 1], f32, tag="r")
                nc.scalar.activation(r[:], ps_r[:], AF.Sigmoid)
                z = sm.tile([h, 1], f32, tag="z")
                nc.scalar.activation(z[:], ps_z[:], AF.Sigmoid)
                t1 = sm.tile([h, 1], f32, tag="t1")
                nc.vector.tensor_mul(t1[:], r[:], ps_hn[:])
                nc.vector.tensor_add(t1[:], t1[:], ps_in[:])
                n_t = sm.tile([h, 1], f32, tag="nt")
                nc.scalar.activation(n_t[:], t1[:], AF.Tanh)
                hmn = sm.tile([h, 1], f32, tag="hmn")
                nc.vector.tensor_sub(hmn[:], haug[0:h, :], n_t[:])
                nc.vector.tensor_mul(hmn[:], hmn[:], z[:])
                nc.vector.tensor_add(haug[0:h, :], n_t[:], hmn[:])

            ps_o = pspool.tile([8, 1], f32, tag="pso")
            nc.tensor.matmul(ps_o[:], hw_s[:], haug[:], start=True, stop=True)
            o = sm.tile([8, 1], f32, tag="o")
            nc.scalar.activation(o[:], ps_o[:], AF.Sigmoid)
            nc.sync.dma_start(out_d[:], o[:])
    nc.compile()
    return nc


_P1_CACHE = {}
_P2_CACHE = {}

# Dev/profiling knobs (test.py pokes these; harness leaves defaults).
TRACE = False
LAST_RES = {}


def _get_phase1(plan):
    key = plan.key()
    if key not in _P1_CACHE:
        nc = bacc.Bacc("TRN2", target_bir_lowering=False, debug=False,
                       num_devices=T)
        _P1_CACHE[key] = build_phase1(nc, plan)
    return _P1_CACHE[key]


def _get_phase2():
    key = (T, H)
    if key not in _P2_CACHE:
        nc = bacc.Bacc("TRN2", target_bir_lowering=False, debug=False,
                       num_devices=1)
        _P2_CACHE[key] = build_phase2(nc, T, H)
    return _P2_CACHE[key]


def kernel(x, edge_index, edge_weight, W1, b1, W2, b2, Wih, Whh, bih, bhh,
           headW, headb):
    x = np.asarray(x, np.float32)
    edge_index = np.asarray(edge_index)
    edge_weight = np.asarray(edge_weight, np.float32)
    W1 = np.asarray(W1, np.float32)
    b1 = np.asarray(b1, np.float32)
    W2 = np.asarray(W2, np.float32)
    b2 = np.asarray(b2, np.float32)

    plan = make_plan([edge_index[t, 1] for t in range(T)], N, E, F, H)
    nc1 = _get_phase1(plan)

    b1B = np.ascontiguousarray(np.broadcast_to(b1, (P, H)), dtype=np.float32)
    b2B = np.ascontiguousarray(np.broadcast_to(b2, (P, H)), dtype=np.float32)
    ident = np.eye(P, dtype=np.float32)
    in_maps = []
    for t in range(T):
        m = graph_inputs(plan, x[t], edge_index[t], edge_weight[t])
        m.update({"W1": W1, "W2": W2, "b1B": b1B, "b2B": b2B, "ident": ident})
        in_maps.append(m)

    res1 = bass_utils.run_bass_kernel_spmd(nc1, in_maps,
                                           core_ids=list(range(T)),
                                           trace=TRACE)
    LAST_RES["p1"] = res1
    seq = np.stack([np.asarray(res1.results[t]["india"]).reshape(H)
                    for t in range(T)])

    nc2 = _get_phase2()
    wih_a = np.concatenate([np.asarray(Wih, np.float32).T,
                            np.asarray(bih, np.float32)[None, :]], axis=0)
    whh_a = np.concatenate([np.asarray(Whh, np.float32).T,
                            np.asarray(bhh, np.float32)[None, :]], axis=0)
    hw_a = np.concatenate([np.asarray(headW, np.float32).T,
                           np.asarray(headb, np.float32)[None, :]], axis=0)
    in2 = [{"seqT": np.ascontiguousarray(seq.T), "WihTa": wih_a,
            "WhhTa": whh_a, "headWTa": hw_a}]
    res2 = bass_utils.run_bass_kernel_spmd(nc2, in2, core_ids=[0],
                                           trace=TRACE)
    LAST_RES["p2"] = res2
    return np.asarray(res2.results[0]["out"]).reshape(8).astype(np.float32)



# revision 4
# speedup vs baseline: 28.8875x; 28.8875x over previous
"""Trainium2 Bass kernel for nn_SanctionImpactGNN.

Temporal GNN: per timestep t (T=8) a 2-layer GCN over a 20000-node /
320000-edge graph; node-0 ("india") embeddings over time feed a tiny GRU +
sigmoid heads -> [8] output.

Key observation
---------------
The reference returns only h2[india] per graph.  That value depends solely on
node 0's 2-hop in-neighborhood:

  * D  = {0} u in-neighbors(0)           (~15-20 nodes)   -- layer-1 outputs
  * A  = D u in-neighbors(D)             (~250-350 nodes) -- layer-1 sources
  * layer-1 edges: all edges with dst in D (~300)
  * layer-2 edges: all edges with dst = 0 (~15-20)
  * degrees (for the symmetric GCN norm) of every node in A, which need the
    full in-edge weight lists of those nodes (~5000 edge weights).

Everything else in the graph is dead code w.r.t. the output, so the kernel
computes exactly this subgraph.  The host does *index* work only (masking,
packing, permutation); every floating-point operation stays on device.

Per-core layout (one graph snapshot per NeuronCore, data-parallel over T):
  * x[A]^T, per-node in-edge weight slots, and dense (layered) weighted
    adjacency matrices M1 (layer-1, A -> D) and M3 (layer-2, D -> {0}) are
    packed into two fp32 blobs and DMAed in two transfers.
  * deg = 1 + rowsum(ew slots); dis = 1/sqrt(deg)              (DVE+Act)
  * g1 = dis * (x[A] @ W1)                                     (PE + DVE)
  * h1 = relu(dis_D * (M1^T @ g1) + b1)   [dup edges + self-loops split
    across L1 "layers" of M1 so the host never sums weights]   (PE + DVE+Act)
  * g2 = dis_D * ((h1 @ W2))                                   (PE + DVE)
  * h2[0] = relu(dis_0 * (M3^T @ g2) + b2) -> india embedding  (PE + DVE+Act)

Phase 2 (single core): 8-step GRU + sigmoid heads with biases folded in via
augmented-ones rows; gi for all steps precomputed in 3 matmuls; per-step gate
math fused into Act ops (sigmoid/tanh with AP scale/bias).
"""

import numpy as np

import concourse.bacc as bacc
import concourse.mybir as mybir
import concourse.tile as tile
from concourse import bass_utils

F32 = mybir.dt.float32
AF = mybir.ActivationFunctionType
OP = mybir.AluOpType
AX = mybir.AxisListType

# Problem constants (hardcoded per contest contract).
T, N, E, F, H = 8, 20000, 320000, 128, 64
P = 128
INDIA = 0
CAP_D = 64  # max |{0} u in-neighbors(0)| supported (observed ~20)


def _analyze(src, dst, w):
    """Pure-index extraction of node 0's 2-hop in-neighborhood."""
    m0 = dst == INDIA
    s1 = np.unique(src[m0])
    D = np.concatenate([[INDIA], s1[s1 != INDIA]]).astype(np.int64)
    nD = len(D)
    assert nD <= CAP_D, f"|D|={nD} exceeds CAP_D={CAP_D}"
    mDe = np.isin(dst, D)
    extra = np.setdiff1d(np.unique(src[mDe]), D)
    A = np.concatenate([D, extra.astype(np.int64)])
    nA = len(A)
    pos = np.full(N, -1, np.int64)
    pos[A] = np.arange(nA)

    # per-A-node in-edge weight slots (for degrees)
    mA = np.isin(dst, A)
    rdeg = pos[dst[mA]]
    o = np.argsort(rdeg, kind="stable")
    rdeg = rdeg[o]
    kdeg = np.arange(len(rdeg)) - np.searchsorted(rdeg, rdeg, "left")
    vdeg = np.asarray(w[mA], np.float32)[o]

    # layer-1 weighted adjacency entries (incl. unit self-loops), layered so
    # coincident (src,dst) cells never need host-side summation
    sM = np.concatenate([pos[src[mDe]], np.arange(nD)])
    dM = np.concatenate([pos[dst[mDe]], np.arange(nD)])
    vM = np.concatenate([np.asarray(w[mDe], np.float32),
                         np.ones(nD, np.float32)])
    assert (sM >= 0).all() and (dM >= 0).all() and (dM < nD).all()
    key = sM * CAP_D + dM
    o1 = np.argsort(key, kind="stable")
    ks = key[o1]
    lM = np.arange(len(ks)) - np.searchsorted(ks, ks, "left")

    # layer-2 entries: edges into node 0 (+ its self-loop)
    s3 = np.concatenate([pos[src[m0]], [0]])
    v3 = np.concatenate([np.asarray(w[m0], np.float32),
                         np.ones(1, np.float32)])
    assert (s3 >= 0).all() and (s3 < nD).all()
    o3 = np.argsort(s3, kind="stable")
    s3 = s3[o3]
    l3 = np.arange(len(s3)) - np.searchsorted(s3, s3, "left")

    return dict(A=A, nA=nA, nD=nD,
                deg_r=rdeg, deg_k=kdeg, deg_v=vdeg,
                m1_s=sM[o1], m1_d=dM[o1], m1_l=lM, m1_v=vM[o1],
                m3_s=s3, m3_l=l3, m3_v=v3[o3])


def _dims_from(infos):
    G = max(1, -(-max(i["nA"] for i in infos) // P))
    capdeg = max(8, int(max(i["deg_k"].max() + 1 if len(i["deg_k"]) else 1
                            for i in infos)))
    capdeg = (capdeg + 3) // 4 * 4
    L1 = int(max(i["m1_l"].max() + 1 for i in infos))
    L3 = int(max(i["m3_l"].max() + 1 for i in infos))
    return G, capdeg, L1, L3


def _fill_blobs(info, x_t, W1, W2, b1, b2, dims):
    G, capdeg, L1, L3 = dims
    nA = info["nA"]
    xT = np.zeros((P, G * P), np.float32)
    xT[:, :nA] = np.asarray(x_t, np.float32)[info["A"]].T
    ewdeg = np.zeros((P, G, capdeg), np.float32)
    ewdeg[info["deg_r"] % P, info["deg_r"] // P, info["deg_k"]] = info["deg_v"]
    m1 = np.zeros((P, L1, G, CAP_D), np.float32)
    m1[info["m1_s"] % P, info["m1_l"], info["m1_s"] // P,
       info["m1_d"]] = info["m1_v"]
    m3 = np.zeros((CAP_D, L3, 4), np.float32)
    m3[info["m3_s"], info["m3_l"], 0] = info["m3_v"]

    blob1 = np.concatenate(
        [xT, ewdeg.reshape(P, -1), np.asarray(W1, np.float32)], axis=1)

    def pad128(a):
        a = np.asarray(a, np.float32)
        return np.concatenate([a, np.zeros((P - a.shape[0], a.shape[1]),
                                           np.float32)], axis=0)

    blob2 = np.concatenate(
        [m1.reshape(P, -1),
         pad128(m3.reshape(CAP_D, -1)),
         pad128(np.asarray(W2, np.float32)),
         pad128(np.eye(CAP_D, dtype=np.float32)),
         pad128(np.broadcast_to(np.asarray(b1, np.float32), (CAP_D, H))),
         pad128(np.broadcast_to(np.asarray(b2, np.float32), (CAP_D, H)))],
        axis=1)
    return {"blob1": np.ascontiguousarray(blob1),
            "blob2": np.ascontiguousarray(blob2)}


def build_phase1(nc, dims):
    G, capdeg, L1, L3 = dims
    nb1 = G * P + G * capdeg + H
    nb2 = L1 * G * CAP_D + L3 * 4 + H + CAP_D + H + H

    b1_d = nc.dram_tensor("blob1", [P, nb1], F32, kind="ExternalInput")
    b2_d = nc.dram_tensor("blob2", [P, nb2], F32, kind="ExternalInput")
    india_d = nc.dram_tensor("india", [1, H], F32, kind="ExternalOutput")

    with tile.TileContext(nc) as tc:
        with (
            tc.tile_pool(name="const", bufs=1) as const,
            tc.tile_pool(name="sm", bufs=8) as sm,
            tc.tile_pool(name="ps", bufs=2, space="PSUM") as pspool,
            tc.tile_pool(name="ps2", bufs=1, space="PSUM") as pspool2,
        ):
            b1t = const.tile([P, nb1], F32, tag="b1t")
            b2t = const.tile([P, nb2], F32, tag="b2t")
            nc.sync.dma_start(b1t[:], b1_d[:])
            nc.sync.dma_start(b2t[:], b2_d[:])

            xT = b1t[:, 0:G * P]
            ewd = b1t[:, G * P:G * P + G * capdeg].rearrange(
                "p (g c) -> p g c", c=capdeg)
            w1v = b1t[:, G * P + G * capdeg:nb1]
            o = L1 * G * CAP_D
            m1t = b2t[:, 0:o]
            m3t = b2t[0:CAP_D, o:o + L3 * 4]
            o += L3 * 4
            w2v = b2t[0:CAP_D, o:o + H]
            o += H
            idv = b2t[0:CAP_D, o:o + CAP_D]
            o += CAP_D
            b1b = b2t[0:CAP_D, o:o + H]
            o += H
            b2b = b2t[0:CAP_D, o:o + H]

            deg = sm.tile([P, G], F32, tag="deg")
            dis = sm.tile([P, G], F32, tag="dis")
            nc.vector.reduce_sum(deg[:], ewd, axis=AX.X)
            nc.scalar.activation(deg[:], deg[:], AF.Sqrt, bias=1.0)
            nc.vector.reciprocal(dis[:], deg[:])

            # g1 = dis * (x[A] @ W1)
            g1 = const.tile([P, G * H], F32, tag="g1")
            for g in range(G):
                ps = pspool.tile([P, H], F32, tag="ps")
                nc.tensor.matmul(ps[:], xT[:, g * P:(g + 1) * P], w1v,
                                 start=True, stop=True)
                nc.vector.tensor_scalar_mul(g1[:, g * H:(g + 1) * H], ps[:],
                                            dis[:, g:g + 1])

            # layer-1 aggregation: h1 = relu(dis_D * (M1^T @ g1) + b1)
            ps1 = pspool2.tile([CAP_D, H], F32, tag="ps1")
            k, nmm = 0, L1 * G
            for l in range(L1):
                for g in range(G):
                    nc.tensor.matmul(ps1[:],
                                     m1t[:, (l * G + g) * CAP_D:
                                         (l * G + g + 1) * CAP_D],
                                     g1[:, g * H:(g + 1) * H],
                                     start=(k == 0), stop=(k == nmm - 1))
                    k += 1
            t1 = sm.tile([CAP_D, H], F32, tag="t1")
            nc.vector.scalar_tensor_tensor(t1[:], ps1[:], dis[0:CAP_D, 0:1],
                                           b1b, OP.mult, OP.add)
            h1 = sm.tile([CAP_D, H], F32, tag="h1")
            nc.scalar.activation(h1[:], t1[:], AF.Relu)

            # g2 = dis_D * (h1 @ W2)
            pst = pspool2.tile([CAP_D, CAP_D], F32, tag="pst")
            nc.tensor.transpose(pst[:], h1[:], idv)
            h1t = sm.tile([CAP_D, CAP_D], F32, tag="h1t")
            nc.vector.tensor_copy(h1t[:], pst[:])
            ps2 = pspool2.tile([CAP_D, H], F32, tag="ps2")
            nc.tensor.matmul(ps2[:], h1t[:], w2v, start=True, stop=True)
            g2 = sm.tile([CAP_D, H], F32, tag="g2")
            nc.vector.tensor_scalar_mul(g2[:], ps2[:], dis[0:CAP_D, 0:1])

            # layer-2 aggregation for node 0 only
            ps3 = pspool2.tile([CAP_D, H], F32, tag="ps3")
            for l in range(L3):
                nc.tensor.matmul(ps3[0:4, :], m3t[:, l * 4:(l + 1) * 4], g2[:],
                                 start=(l == 0), stop=(l == L3 - 1))
            t2 = sm.tile([CAP_D, H], F32, tag="t2")
            nc.vector.scalar_tensor_tensor(t2[0:1, :], ps3[0:1, :],
                                           dis[0:1, 0:1], b2b[0:1, :],
                                           OP.mult, OP.add)
            out_t = sm.tile([CAP_D, H], F32, tag="outt")
            nc.scalar.activation(out_t[0:1, :], t2[0:1, :], AF.Relu)
            nc.sync.dma_start(india_d[:], out_t[0:1, :])
    nc.compile()
    return nc


def build_phase2(nc, t_steps, h):
    f32 = F32
    seq_d = nc.dram_tensor("seqT", [h, t_steps], f32, kind="ExternalInput")
    wih_d = nc.dram_tensor("WihTa", [h + 1, 3 * h], f32, kind="ExternalInput")
    whh_d = nc.dram_tensor("WhhTa", [h + 1, 3 * h], f32, kind="ExternalInput")
    hw_d = nc.dram_tensor("headWTa", [h + 1, 8], f32, kind="ExternalInput")
    out_d = nc.dram_tensor("out", [8, 1], f32, kind="ExternalOutput")

    with tile.TileContext(nc) as tc:
        with (
            tc.tile_pool(name="const", bufs=1) as const,
            tc.tile_pool(name="sm", bufs=6) as sm,
            tc.tile_pool(name="ps", bufs=1, space="PSUM") as pspool,
        ):
            wih_s = const.tile([h + 1, 3 * h], f32, tag="wih")
            whh_s = const.tile([h + 1, 3 * h], f32, tag="whh")
            hw_s = const.tile([h + 1, 8], f32, tag="hw")
            xaug = const.tile([h + 1, t_steps], f32, tag="xaug")
            haug = const.tile([h + 1, 1], f32, tag="haug")

            nc.sync.dma_start(wih_s[:], wih_d[:])
            nc.sync.dma_start(whh_s[:], whh_d[:])
            nc.sync.dma_start(hw_s[:], hw_d[:])
            nc.sync.dma_start(xaug[0:h, :], seq_d[:])
            nc.vector.memset(xaug[h:h + 1, :], 1.0)
            nc.vector.memset(haug[0:h, :], 0.0)
            nc.vector.memset(haug[h:h + 1, :], 1.0)

            # gi = Wih^T x_t + bih for all steps, one matmul per gate
            gi = []
            for j, tag in enumerate(("gir", "giz", "gin")):
                psg = pspool.tile([h, t_steps], f32, tag="psg")
                nc.tensor.matmul(psg[:], wih_s[:, j * h:(j + 1) * h], xaug[:],
                                 start=True, stop=True)
                gt = const.tile([h, t_steps], f32, tag=tag)
                nc.vector.tensor_copy(gt[:], psg[:])
                gi.append(gt)
            gir, giz, gin = gi

            for t in range(t_steps):
                psr = pspool.tile([h, 1], f32, tag="psr")
                nc.tensor.matmul(psr[:], whh_s[:, 0:h], haug[:],
                                 start=True, stop=True)
                psz = pspool.tile([h, 1], f32, tag="psz")
                nc.tensor.matmul(psz[:], whh_s[:, h:2 * h], haug[:],
                                 start=True, stop=True)
                psn = pspool.tile([h, 1], f32, tag="psn")
                nc.tensor.matmul(psn[:], whh_s[:, 2 * h:3 * h], haug[:],
                                 start=True, stop=True)
                r = sm.tile([h, 1], f32, tag="r")
                nc.scalar.activation(r[:], psr[:], AF.Sigmoid,
                                     bias=gir[:, t:t + 1])
                z = sm.tile([h, 1], f32, tag="z")
                nc.scalar.activation(z[:], psz[:], AF.Sigmoid,
                                     bias=giz[:, t:t + 1])
                n_t = sm.tile([h, 1], f32, tag="nt")
                nc.scalar.activation(n_t[:], psn[:], AF.Tanh,
                                     bias=gin[:, t:t + 1], scale=r[:])
                hm = sm.tile([h, 1], f32, tag="hm")
                nc.vector.tensor_sub(hm[:], haug[0:h, :], n_t[:])
                nc.vector.scalar_tensor_tensor(haug[0:h, :], hm[:], z[:],
                                               n_t[:], OP.mult, OP.add)

            ps_o = pspool.tile([8, 1], f32, tag="pso")
            nc.tensor.matmul(ps_o[:], hw_s[:], haug[:], start=True, stop=True)
            o = sm.tile([8, 1], f32, tag="o")
            nc.scalar.activation(o[:], ps_o[:], AF.Sigmoid)
            nc.sync.dma_start(out_d[:], o[:])
    nc.compile()
    return nc


_P1_CACHE = {}
_P2_CACHE = {}

# Dev/profiling knobs (test.py pokes these; harness leaves defaults).
TRACE = False
LAST_RES = {}


def _get_phase1(dims):
    key = tuple(dims)
    if key not in _P1_CACHE:
        nc = bacc.Bacc("TRN2", target_bir_lowering=False, debug=False,
                       num_devices=T)
        _P1_CACHE[key] = build_phase1(nc, dims)
    return _P1_CACHE[key]


def _get_phase2():
    key = (T, H)
    if key not in _P2_CACHE:
        nc = bacc.Bacc("TRN2", target_bir_lowering=False, debug=False,
                       num_devices=1)
        _P2_CACHE[key] = build_phase2(nc, T, H)
    return _P2_CACHE[key]


def kernel(x, edge_index, edge_weight, W1, b1, W2, b2, Wih, Whh, bih, bhh,
           headW, headb):
    x = np.asarray(x, np.float32)
    edge_index = np.asarray(edge_index)
    edge_weight = np.asarray(edge_weight, np.float32)

    infos = [_analyze(np.asarray(edge_index[t, 0]),
                      np.asarray(edge_index[t, 1]), edge_weight[t])
             for t in range(T)]
    dims = _dims_from(infos)
    nc1 = _get_phase1(dims)

    in_maps = [_fill_blobs(infos[t], x[t], W1, W2, b1, b2, dims)
               for t in range(T)]
    res1 = bass_utils.run_bass_kernel_spmd(nc1, in_maps,
                                           core_ids=list(range(T)),
                                           trace=TRACE)
    LAST_RES["p1"] = res1
    seq = np.stack([np.asarray(res1.results[t]["india"]).reshape(H)
                    for t in range(T)])

    nc2 = _get_phase2()
    wih_a = np.concatenate([np.asarray(Wih, np.float32).T,
                            np.asarray(bih, np.float32)[None, :]], axis=0)
    whh_a = np.concatenate([np.asarray(Whh, np.float32).T,
                            np.asarray(bhh, np.float32)[None, :]], axis=0)
    hw_a = np.concatenate([np.asarray(headW, np.float32).T,
                           np.asarray(headb, np.float32)[None, :]], axis=0)
    in2 = [{"seqT": np.ascontiguousarray(seq.T), "WihTa": wih_a,
            "WhhTa": whh_a, "headWTa": hw_a}]
    res2 = bass_utils.run_bass_kernel_spmd(nc2, in2, core_ids=[0],
                                           trace=TRACE)
    LAST_RES["p2"] = res2
    return np.asarray(res2.results[0]["out"]).reshape(8).astype(np.float32)


# revision 9
# speedup vs baseline: 36.0481x; 1.2479x over previous
"""Trainium2 Bass kernel for nn_SanctionImpactGNN.

Temporal GNN: per timestep t (T=8) a 2-layer GCN over a 20000-node /
320000-edge graph; node-0 ("india") embeddings over time feed a tiny GRU +
sigmoid heads -> [8] output.

Key observation
---------------
The reference returns only h2[india] per graph.  That value depends solely on
node 0's 2-hop in-neighborhood:

  * D  = {0} u in-neighbors(0)           (~15-20 nodes)   -- layer-1 outputs
  * A  = D u in-neighbors(D)             (~250-350 nodes) -- layer-1 sources
  * layer-1 edges: all edges with dst in D (~300)
  * layer-2 edges: all edges with dst = 0 (~15-20)
  * degrees (for the symmetric GCN norm) of every node in A, which need the
    full in-edge weight lists of those nodes (~5000 edge weights).

Everything else in the graph is dead code w.r.t. the output, so the kernel
computes exactly this subgraph.  The host does *index* work only (masking,
packing, permutation, dtype packing); every floating-point operation stays on
device.

Per-core (one graph snapshot per NeuronCore, data-parallel over T):
  * deg_A = 1 + rowsum(ew slots)  -> dis_A = 1/sqrt(deg_A)   [partition axis]
  * deg_D via ones-matmul on a transposed slot pack -> dis as a row, expanded
    to disX[h,d] by a rank-1 matmul (keeps every dis scale off the critical
    path and avoids any PE transpose of activations)
  * g1 = dis_A * (x[A] @ W1)
  * h1^T = relu((g1^T-contracted M1 matmul) * disX + b1)   [M1 layered dense
    weighted adjacency; duplicate edges/self-loop collisions get their own
    layer so the host never sums weights]
  * g2 = h1 @ W2;  h2[0] = relu(dis_0 * (M3s^T @ g2) + b2)  [M3s rows are
    dis_D-scaled on device]
All matmuls run in fp16 (inputs quantized host-side; PSUM accumulates fp32).

Phase 2 (single core): 8-step GRU + sigmoid heads, biases folded via
augmented-ones rows, one fp16 blob load, gi for all steps precomputed, gate
math fused into Act ops (sigmoid/tanh with AP scale/bias).
"""

import numpy as np

import concourse.bacc as bacc
import concourse.mybir as mybir
import concourse.tile as tile
from concourse import bass_utils

F32 = mybir.dt.float32
F16 = mybir.dt.float16
AF = mybir.ActivationFunctionType
OP = mybir.AluOpType
AX = mybir.AxisListType

# Problem constants (hardcoded per contest contract).
T, N, E, F, H = 8, 20000, 320000, 128, 64
P = 128
INDIA = 0
CAP_D = 32  # max |{0} u in-neighbors(0)| supported (observed ~20)


def _analyze(src, dst, w):
    """Pure-index extraction of node 0's 2-hop in-neighborhood."""
    m0 = dst == INDIA
    s1 = np.unique(src[m0])
    D = np.concatenate([[INDIA], s1[s1 != INDIA]]).astype(np.int64)
    nD = len(D)
    assert nD <= CAP_D, f"|D|={nD} exceeds CAP_D={CAP_D}"
    mDe = np.isin(dst, D)
    extra = np.setdiff1d(np.unique(src[mDe]), D)
    A = np.concatenate([D, extra.astype(np.int64)])
    nA = len(A)
    pos = np.full(N, -1, np.int64)
    pos[A] = np.arange(nA)

    # per-A-node in-edge weight slots (partition-axis degree layout)
    mA = np.isin(dst, A)
    rdeg = pos[dst[mA]]
    o = np.argsort(rdeg, kind="stable")
    rdeg = rdeg[o]
    kdeg = np.arange(len(rdeg)) - np.searchsorted(rdeg, rdeg, "left")
    vdeg = np.asarray(w[mA], np.float32)[o]

    # layer-1 weighted adjacency entries (incl. unit self-loops), layered so
    # coincident (src,dst) cells never need host-side summation
    sM = np.concatenate([pos[src[mDe]], np.arange(nD)])
    dM = np.concatenate([pos[dst[mDe]], np.arange(nD)])
    vM = np.concatenate([np.asarray(w[mDe], np.float32),
                         np.ones(nD, np.float32)])
    assert (sM >= 0).all() and (dM >= 0).all() and (dM < nD).all()
    key = sM * CAP_D + dM
    o1 = np.argsort(key, kind="stable")
    ks = key[o1]
    lM = np.arange(len(ks)) - np.searchsorted(ks, ks, "left")

    # layer-2 entries: edges into node 0 (+ its self-loop)
    s3 = np.concatenate([pos[src[m0]], [0]])
    v3 = np.concatenate([np.asarray(w[m0], np.float32),
                         np.ones(1, np.float32)])
    assert (s3 >= 0).all() and (s3 < nD).all()
    o3 = np.argsort(s3, kind="stable")
    s3 = s3[o3]
    l3 = np.arange(len(s3)) - np.searchsorted(s3, s3, "left")

    return dict(A=A, nA=nA, nD=nD,
                deg_r=rdeg, deg_k=kdeg, deg_v=vdeg,
                m1_s=sM[o1], m1_d=dM[o1], m1_l=lM, m1_v=vM[o1],
                m3_s=s3, m3_l=l3, m3_v=v3[o3])


def _dims_from(infos):
    G = max(1, -(-max(i["nA"] for i in infos) // P))
    capdeg = max(8, int(max(i["deg_k"].max() + 1 if len(i["deg_k"]) else 1
                            for i in infos)))
    capdeg = (capdeg + 3) // 4 * 4
    L1 = int(max(i["m1_l"].max() + 1 for i in infos))
    L3 = int(max(i["m3_l"].max() + 1 for i in infos))
    return G, capdeg, L1, L3


def _blob1_offsets(dims):
    G, capdeg, L1, L3 = dims
    o = {}
    c = 0
    for name, width in (("ewdeg", G * capdeg), ("xT", G * P), ("w1", H),
                        ("ewD", max(capdeg, CAP_D)), ("ones_c", 1), ("ones_r", H)):
        o[name] = c
        c += width
    return o, c


def _blob2_offsets(dims):
    G, capdeg, L1, L3 = dims
    o = {}
    c = 0
    for name, width in (("m1", L1 * G * CAP_D), ("m3", L3 * 4), ("w2", H),
                        ("b1c", 1), ("b2r", H)):
        o[name] = c
        c += width
    return o, c


def _fill_blobs(info, x_t, W1, W2, b1, b2, dims):
    G, capdeg, L1, L3 = dims
    nA, nD = info["nA"], info["nD"]
    o1, nb1 = _blob1_offsets(dims)
    o2, nb2 = _blob2_offsets(dims)
    blob1 = np.zeros((P, nb1), np.float16)
    blob2 = np.zeros((P, nb2), np.float16)

    ewdeg = np.zeros((P, G, capdeg), np.float16)
    ewdeg[info["deg_r"] % P, info["deg_r"] // P, info["deg_k"]] = info["deg_v"]
    blob1[:, o1["ewdeg"]:o1["ewdeg"] + G * capdeg] = ewdeg.reshape(P, -1)
    blob1[:, o1["xT"]:o1["xT"] + nA] = \
        np.asarray(x_t, np.float32)[info["A"]].T.astype(np.float16)
    blob1[:, o1["w1"]:o1["w1"] + H] = np.asarray(W1, np.float16)
    # transposed in-edge slots for D nodes only (slot on partition, node on
    # free) -> degrees of D as a row via ones-matmul
    mD = info["deg_r"] < nD
    blob1[info["deg_k"][mD], o1["ewD"] + info["deg_r"][mD]] = \
        info["deg_v"][mD].astype(np.float16)
    blob1[0:capdeg, o1["ones_c"]] = 1.0
    blob1[0, o1["ones_r"]:o1["ones_r"] + H] = 1.0

    m1 = np.zeros((P, L1, G, CAP_D), np.float16)
    m1[info["m1_s"] % P, info["m1_l"], info["m1_s"] // P,
       info["m1_d"]] = info["m1_v"]
    blob2[:, o2["m1"]:o2["m1"] + L1 * G * CAP_D] = m1.reshape(P, -1)
    m3 = np.zeros((CAP_D, L3, 4), np.float16)
    m3[info["m3_s"], info["m3_l"], 0] = info["m3_v"]
    blob2[0:CAP_D, o2["m3"]:o2["m3"] + L3 * 4] = m3.reshape(CAP_D, -1)
    blob2[0:H, o2["w2"]:o2["w2"] + H] = np.asarray(W2, np.float16)
    blob2[0:H, o2["b1c"]] = np.asarray(b1, np.float16)
    blob2[0, o2["b2r"]:o2["b2r"] + H] = np.asarray(b2, np.float16)
    return {"blob1": blob1, "blob2": blob2}


def build_phase1(nc, dims):
    G, capdeg, L1, L3 = dims
    o1, nb1 = _blob1_offsets(dims)
    o2, nb2 = _blob2_offsets(dims)

    b1_d = nc.dram_tensor("blob1", [P, nb1], F16, kind="ExternalInput")
    b2_d = nc.dram_tensor("blob2", [P, nb2], F16, kind="ExternalInput")
    india_d = nc.dram_tensor("india", [1, H], F32, kind="ExternalOutput")

    with tile.TileContext(nc) as tc:
        with (
            tc.tile_pool(name="const", bufs=1) as const,
            tc.tile_pool(name="sm", bufs=8) as sm,
            tc.tile_pool(name="psa", bufs=3, space="PSUM") as psa,
            tc.tile_pool(name="psb", bufs=1, space="PSUM") as psb,
        ):
            b1t = const.tile([P, nb1], F16, tag="b1t")
            b2t = const.tile([P, nb2], F16, tag="b2t")
            nc.sync.dma_start(b1t[:], b1_d[:])
            nc.sync.dma_start(b2t[:], b2_d[:])

            ewd3 = b1t[:, o1["ewdeg"]:o1["ewdeg"] + G * capdeg].rearrange(
                "p (g c) -> p g c", c=capdeg)
            xTv = b1t[:, o1["xT"]:o1["xT"] + G * P]
            w1v = b1t[:, o1["w1"]:o1["w1"] + H]
            ewD = b1t[0:capdeg, o1["ewD"]:o1["ewD"] + capdeg]
            ones_c = b1t[0:capdeg, o1["ones_c"]:o1["ones_c"] + 1]
            ones_r = b1t[0:1, o1["ones_r"]:o1["ones_r"] + H]
            m1v = b2t[:, o2["m1"]:o2["m1"] + L1 * G * CAP_D]
            m3v = b2t[0:CAP_D, o2["m3"]:o2["m3"] + L3 * 4]
            w2v = b2t[0:H, o2["w2"]:o2["w2"] + H]
            b1c = b2t[0:H, o2["b1c"]:o2["b1c"] + 1]
            b2r = b2t[0:1, o2["b2r"]:o2["b2r"] + H]

            # dis over all A nodes (partition layout) for the g1 scale
            deg = sm.tile([P, G], F32, tag="deg")
            dis = sm.tile([P, G], F32, tag="dis")
            nc.vector.reduce_sum(deg[:], ewd3, axis=AX.X)
            nc.scalar.activation(deg[:], deg[:], AF.Sqrt, bias=1.0)
            nc.vector.reciprocal(dis[:], deg[:])

            # dis over D as a row -> disX[h, d] = dis_d (rank-1 matmul)
            psdr = psb.tile([1, CAP_D], F32, tag="psdr")
            nc.tensor.matmul(psdr[:], ones_c, ewD[:, 0:CAP_D],
                             start=True, stop=True)
            sqr = sm.tile([1, CAP_D], F32, tag="sqr")
            nc.scalar.activation(sqr[:], psdr[:], AF.Sqrt, bias=1.0)
            dr = sm.tile([1, CAP_D], F32, tag="dr")
            nc.vector.reciprocal(dr[:], sqr[:])
            drh = sm.tile([1, CAP_D], F16, tag="drh")
            nc.vector.tensor_copy(drh[:], dr[:])
            psdx = psb.tile([H, CAP_D], F32, tag="psdx")
            nc.tensor.matmul(psdx[:], ones_r, drh[:], start=True, stop=True)
            disX = sm.tile([H, CAP_D], F16, tag="disX")
            nc.vector.tensor_copy(disX[:], psdx[:])

            # g1 = dis_A * (x[A] @ W1)
            g1 = const.tile([P, G * H], F16, tag="g1")
            for g in range(G):
                psg = psa.tile([P, H], F32, tag="psg")
                nc.tensor.matmul(psg[:], xTv[:, g * P:(g + 1) * P], w1v,
                                 start=True, stop=True)
                nc.vector.tensor_scalar_mul(g1[:, g * H:(g + 1) * H], psg[:],
                                            dis[:, g:g + 1])

            # dis_D-scaled layer-2 adjacency (off critical path)
            m3s = sm.tile([CAP_D, L3 * 4], F16, tag="m3s")
            nc.vector.tensor_scalar_mul(m3s[:], m3v, dis[0:CAP_D, 0:1])

            # layer-1 aggregation, transposed: ps1t[h, d] = sum_s g1[s,h]M1[s,d]
            ps1t = psb.tile([H, CAP_D], F32, tag="ps1t")
            k, nmm = 0, L1 * G
            for l in range(L1):
                for g in range(G):
                    nc.tensor.matmul(ps1t[:], g1[:, g * H:(g + 1) * H],
                                     m1v[:, (l * G + g) * CAP_D:
                                         (l * G + g + 1) * CAP_D],
                                     start=(k == 0), stop=(k == nmm - 1))
                    k += 1
            b1c32 = sm.tile([H, 1], F32, tag="b1c32")
            nc.vector.tensor_copy(b1c32[:], b1c)
            u = sm.tile([H, CAP_D], F16, tag="u")
            nc.vector.tensor_tensor(u[:], ps1t[:], disX[:], op=OP.mult)
            h1T = sm.tile([H, CAP_D], F16, tag="h1T")
            nc.vector.tensor_scalar(h1T[:], u[:], b1c32[:], 0.0, OP.add, OP.max)

            # layer 2: g2 = h1 @ W2 (dis_D factor lives in m3s)
            ps2 = psb.tile([CAP_D, H], F32, tag="ps2")
            nc.tensor.matmul(ps2[:], h1T[:], w2v, start=True, stop=True)
            g2s = sm.tile([CAP_D, H], F16, tag="g2s")
            nc.vector.tensor_copy(g2s[:], ps2[:])
            ps3 = psb.tile([4, H], F32, tag="ps3")
            for l in range(L3):
                nc.tensor.matmul(ps3[:], m3s[:, l * 4:(l + 1) * 4], g2s[:],
                                 start=(l == 0), stop=(l == L3 - 1))
            t2 = sm.tile([1, H], F32, tag="t2")
            nc.vector.scalar_tensor_tensor(t2[:], ps3[0:1, :], dr[0:1, 0:1],
                                           b2r, OP.mult, OP.add)
            out_t = sm.tile([1, H], F32, tag="outt")
            nc.vector.tensor_scalar_max(out_t[:], t2[:], 0.0)
            nc.sync.dma_start(india_d[:], out_t[:])
    nc.compile()
    return nc


def build_phase2(nc, t_steps, h):
    # column layout: wih [65, 3h] | whh [65, 3h] | hw [65, 8] | xaug [65, T]
    owih, owhh, ohw, oxa = 0, 3 * h, 6 * h, 6 * h + 8
    nbtot = 6 * h + 8 + t_steps
    blob_d = nc.dram_tensor("blob", [h + 1, nbtot], F16, kind="ExternalInput")
    out_d = nc.dram_tensor("out", [8, 1], F32, kind="ExternalOutput")

    with tile.TileContext(nc) as tc:
        with (
            tc.tile_pool(name="const", bufs=1) as const,
            tc.tile_pool(name="sm", bufs=6) as sm,
            tc.tile_pool(name="psa", bufs=3, space="PSUM") as psa,
            tc.tile_pool(name="psb", bufs=1, space="PSUM") as psb,
        ):
            # dummy activation: hoists the (serial) activation-table load to
            # kernel start, off the gi critical path
            dum = sm.tile([1, 1], F32, tag="dum")
            nc.vector.memset(dum[:], 0.0)
            nc.scalar.activation(dum[:], dum[:], AF.Sigmoid)

            bt = const.tile([h + 1, nbtot], F16, tag="bt")
            nc.sync.dma_start(bt[:], blob_d[:])
            wih = bt[:, owih:owih + 3 * h]
            whh = bt[:, owhh:owhh + 3 * h]
            hw = bt[:, ohw:ohw + 8]
            xa = bt[:, oxa:oxa + t_steps]

            haug = const.tile([h + 1, 1], F16, tag="haug")
            nc.vector.memset(haug[0:h, :], 0.0)
            nc.vector.memset(haug[h:h + 1, :], 1.0)

            gi = []
            for j, tag in enumerate(("gir", "giz", "gin")):
                psg = psa.tile([h, t_steps], F32, tag="psg")
                nc.tensor.matmul(psg[:], wih[:, j * h:(j + 1) * h], xa,
                                 start=True, stop=True)
                gt = const.tile([h, t_steps], F16, tag=tag)
                nc.vector.tensor_copy(gt[:], psg[:])
                gi.append(gt)
            gir, giz, gin = gi

            for t in range(t_steps):
                psr = psb.tile([h, 1], F32, tag="psr")
                nc.tensor.matmul(psr[:], whh[:, 0:h], haug[:],
                                 start=True, stop=True)
                psz = psb.tile([h, 1], F32, tag="psz")
                nc.tensor.matmul(psz[:], whh[:, h:2 * h], haug[:],
                                 start=True, stop=True)
                psn = psb.tile([h, 1], F32, tag="psn")
                nc.tensor.matmul(psn[:], whh[:, 2 * h:3 * h], haug[:],
                                 start=True, stop=True)
                r = sm.tile([h, 1], F32, tag="r")
                nc.scalar.activation(r[:], psr[:], AF.Sigmoid,
                                     bias=gir[:, t:t + 1])
                z = sm.tile([h, 1], F32, tag="z")
                nc.scalar.activation(z[:], psz[:], AF.Sigmoid,
                                     bias=giz[:, t:t + 1])
                n_t = sm.tile([h, 1], F16, tag="nt")
                nc.scalar.activation(n_t[:], psn[:], AF.Tanh,
                                     bias=gin[:, t:t + 1], scale=r[:])
                hm = sm.tile([h, 1], F16, tag="hm")
                nc.vector.tensor_sub(hm[:], haug[0:h, :], n_t[:])
                nc.vector.scalar_tensor_tensor(haug[0:h, :], hm[:], z[:],
                                               n_t[:], OP.mult, OP.add)

            ps_o = psb.tile([8, 1], F32, tag="pso")
            nc.tensor.matmul(ps_o[:], hw, haug[:], start=True, stop=True)
            o = sm.tile([8, 1], F32, tag="o")
            nc.scalar.activation(o[:], ps_o[:], AF.Sigmoid)
            nc.sync.dma_start(out_d[:], o[:])
    nc.compile()
    return nc


_P1_CACHE = {}
_P2_CACHE = {}

# Dev/profiling knobs (test.py pokes these; harness leaves defaults).
TRACE = False
LAST_RES = {}


def _get_phase1(dims):
    key = tuple(dims)
    if key not in _P1_CACHE:
        nc = bacc.Bacc("TRN2", target_bir_lowering=False, debug=False,
                       num_devices=T)
        _P1_CACHE[key] = build_phase1(nc, dims)
    return _P1_CACHE[key]


def _get_phase2():
    key = (T, H)
    if key not in _P2_CACHE:
        nc = bacc.Bacc("TRN2", target_bir_lowering=False, debug=False,
                       num_devices=1)
        _P2_CACHE[key] = build_phase2(nc, T, H)
    return _P2_CACHE[key]


def _p2_blob(seq, Wih, Whh, bih, bhh, headW, headb):
    h, t_steps = H, T
    owih, owhh, ohw, oxa = 0, 3 * h, 6 * h, 6 * h + 8
    blob = np.zeros((h + 1, 6 * h + 8 + t_steps), np.float16)
    blob[0:h, owih:owih + 3 * h] = np.asarray(Wih, np.float16).T
    blob[h, owih:owih + 3 * h] = np.asarray(bih, np.float16)
    blob[0:h, owhh:owhh + 3 * h] = np.asarray(Whh, np.float16).T
    blob[h, owhh:owhh + 3 * h] = np.asarray(bhh, np.float16)
    blob[0:h, ohw:ohw + 8] = np.asarray(headW, np.float16).T
    blob[h, ohw:ohw + 8] = np.asarray(headb, np.float16)
    blob[0:h, oxa:oxa + t_steps] = np.asarray(seq, np.float16).T
    blob[h, oxa:oxa + t_steps] = 1.0
    return blob


def kernel(x, edge_index, edge_weight, W1, b1, W2, b2, Wih, Whh, bih, bhh,
           headW, headb):
    x = np.asarray(x, np.float32)
    edge_index = np.asarray(edge_index)
    edge_weight = np.asarray(edge_weight, np.float32)

    infos = [_analyze(np.asarray(edge_index[t, 0]),
                      np.asarray(edge_index[t, 1]), edge_weight[t])
             for t in range(T)]
    dims = _dims_from(infos)
    nc1 = _get_phase1(dims)

    in_maps = [_fill_blobs(infos[t], x[t], W1, W2, b1, b2, dims)
               for t in range(T)]
    res1 = bass_utils.run_bass_kernel_spmd(nc1, in_maps,
                                           core_ids=list(range(T)),
                                           trace=TRACE)
    LAST_RES["p1"] = res1
    seq = np.stack([np.asarray(res1.results[t]["india"]).reshape(H)
                    for t in range(T)])

    nc2 = _get_phase2()
    in2 = [{"blob": _p2_blob(seq, Wih, Whh, bih, bhh, headW, headb)}]
    res2 = bass_utils.run_bass_kernel_spmd(nc2, in2, core_ids=[0],
                                           trace=TRACE)
    LAST_RES["p2"] = res2
    return np.asarray(res2.results[0]["out"]).reshape(8).astype(np.float32)


# revision 12
# speedup vs baseline: 37.6185x; 1.0436x over previous
"""Trainium2 Bass kernel for nn_SanctionImpactGNN.

Temporal GNN: per timestep t (T=8) a 2-layer GCN over a 20000-node /
320000-edge graph; node-0 ("india") embeddings over time feed a tiny GRU +
sigmoid heads -> [8] output.

Key observation
---------------
The reference returns only h2[india] per graph.  That value depends solely on
node 0's 2-hop in-neighborhood:

  * D  = {0} u in-neighbors(0)           (~15-20 nodes)   -- layer-1 outputs
  * A  = D u in-neighbors(D)             (~250-350 nodes) -- layer-1 sources
  * layer-1 edges: all edges with dst in D (~300)
  * layer-2 edges: all edges with dst = 0 (~15-20)
  * degrees (for the symmetric GCN norm) of every node in A, which need the
    full in-edge weight lists of those nodes (~5000 edge weights).

Everything else in the graph is dead code w.r.t. the output, so the kernel
computes exactly this subgraph.  The host does *index* work only (masking,
packing, permutation, dtype packing); every floating-point operation stays on
device.

Per-core (one graph snapshot per NeuronCore, data-parallel over T):
  * deg_A = 1 + rowsum(ew slots)  -> dis_A = 1/sqrt(deg_A)   [partition axis]
  * deg_D via ones-matmul on a transposed slot pack -> dis as a row, expanded
    to disX[h,d] by a rank-1 matmul (keeps every dis scale off the critical
    path and avoids any PE transpose of activations)
  * g1 = dis_A * (x[A] @ W1)
  * h1^T = relu((g1^T-contracted M1 matmul) * disX + b1)   [M1 layered dense
    weighted adjacency; duplicate edges/self-loop collisions get their own
    layer so the host never sums weights]
  * g2 = h1 @ W2;  h2[0] = relu(dis_0 * (M3s^T @ g2) + b2)  [M3s rows are
    dis_D-scaled on device]
All matmuls run in fp16 (inputs quantized host-side; PSUM accumulates fp32).

Phase 2 (single core): 8-step GRU + sigmoid heads, biases folded via
augmented-ones rows, one fp16 blob load, gi for all steps precomputed, gate
math fused into Act ops (sigmoid/tanh with AP scale/bias).
"""

import numpy as np

import concourse.bacc as bacc
import concourse.mybir as mybir
import concourse.tile as tile
from concourse import bass_utils

F32 = mybir.dt.float32
F16 = mybir.dt.float16
AF = mybir.ActivationFunctionType
OP = mybir.AluOpType
AX = mybir.AxisListType

# Problem constants (hardcoded per contest contract).
T, N, E, F, H = 8, 20000, 320000, 128, 64
P = 128
INDIA = 0
CAP_D = 32  # max |{0} u in-neighbors(0)| supported (observed ~20)


def _analyze(src, dst, w):
    """Pure-index extraction of node 0's 2-hop in-neighborhood."""
    m0 = dst == INDIA
    s1 = np.unique(src[m0])
    D = np.concatenate([[INDIA], s1[s1 != INDIA]]).astype(np.int64)
    nD = len(D)
    assert nD <= CAP_D, f"|D|={nD} exceeds CAP_D={CAP_D}"
    mDe = np.isin(dst, D)
    extra = np.setdiff1d(np.unique(src[mDe]), D)
    A = np.concatenate([D, extra.astype(np.int64)])
    nA = len(A)
    pos = np.full(N, -1, np.int64)
    pos[A] = np.arange(nA)

    # per-A-node in-edge weight slots (partition-axis degree layout)
    mA = np.isin(dst, A)
    rdeg = pos[dst[mA]]
    o = np.argsort(rdeg, kind="stable")
    rdeg = rdeg[o]
    kdeg = np.arange(len(rdeg)) - np.searchsorted(rdeg, rdeg, "left")
    vdeg = np.asarray(w[mA], np.float32)[o]

    # layer-1 weighted adjacency entries (incl. unit self-loops), layered so
    # coincident (src,dst) cells never need host-side summation
    sM = np.concatenate([pos[src[mDe]], np.arange(nD)])
    dM = np.concatenate([pos[dst[mDe]], np.arange(nD)])
    vM = np.concatenate([np.asarray(w[mDe], np.float32),
                         np.ones(nD, np.float32)])
    assert (sM >= 0).all() and (dM >= 0).all() and (dM < nD).all()
    key = sM * CAP_D + dM
    o1 = np.argsort(key, kind="stable")
    ks = key[o1]
    lM = np.arange(len(ks)) - np.searchsorted(ks, ks, "left")

    # layer-2 entries: edges into node 0 (+ its self-loop)
    s3 = np.concatenate([pos[src[m0]], [0]])
    v3 = np.concatenate([np.asarray(w[m0], np.float32),
                         np.ones(1, np.float32)])
    assert (s3 >= 0).all() and (s3 < nD).all()
    o3 = np.argsort(s3, kind="stable")
    s3 = s3[o3]
    l3 = np.arange(len(s3)) - np.searchsorted(s3, s3, "left")

    return dict(A=A, nA=nA, nD=nD,
                deg_r=rdeg, deg_k=kdeg, deg_v=vdeg,
                m1_s=sM[o1], m1_d=dM[o1], m1_l=lM, m1_v=vM[o1],
                m3_s=s3, m3_l=l3, m3_v=v3[o3])


def _dims_from(infos):
    G = max(1, -(-max(i["nA"] for i in infos) // P))
    capdeg = max(8, int(max(i["deg_k"].max() + 1 if len(i["deg_k"]) else 1
                            for i in infos)))
    capdeg = (capdeg + 3) // 4 * 4
    L1 = int(max(i["m1_l"].max() + 1 for i in infos))
    L3 = int(max(i["m3_l"].max() + 1 for i in infos))
    return G, capdeg, L1, L3


def _blob1_offsets(dims):
    G, capdeg, L1, L3 = dims
    o = {}
    c = 0
    for name, width in (("ewdeg", G * capdeg), ("xT", G * P), ("w1", H),
                        ("ewD", max(capdeg, CAP_D)), ("ones_c", 1), ("ones_r", H)):
        o[name] = c
        c += width
    return o, c


def _blob2_offsets(dims):
    G, capdeg, L1, L3 = dims
    o = {}
    c = 0
    for name, width in (("m1", L1 * G * CAP_D), ("m3", L3 * CAP_D),
                        ("w2", H), ("b1c", 1)):
        o[name] = c
        c += width
    return o, c


def _fill_blobs(info, x_t, W1, W2, b1, b2, dims):
    G, capdeg, L1, L3 = dims
    nA, nD = info["nA"], info["nD"]
    o1, nb1 = _blob1_offsets(dims)
    o2, nb2 = _blob2_offsets(dims)
    blob1 = np.zeros((P, nb1), np.float16)
    blob2 = np.zeros((P, nb2), np.float16)

    ewdeg = np.zeros((P, G, capdeg), np.float16)
    ewdeg[info["deg_r"] % P, info["deg_r"] // P, info["deg_k"]] = info["deg_v"]
    blob1[:, o1["ewdeg"]:o1["ewdeg"] + G * capdeg] = ewdeg.reshape(P, -1)
    blob1[:, o1["xT"]:o1["xT"] + nA] = \
        np.asarray(x_t, np.float32)[info["A"]].T.astype(np.float16)
    blob1[:, o1["w1"]:o1["w1"] + H] = np.asarray(W1, np.float16)
    # transposed in-edge slots for D nodes only (slot on partition, node on
    # free) -> degrees of D as a row via ones-matmul
    mD = info["deg_r"] < nD
    blob1[info["deg_k"][mD], o1["ewD"] + info["deg_r"][mD]] = \
        info["deg_v"][mD].astype(np.float16)
    blob1[0:capdeg, o1["ones_c"]] = 1.0
    blob1[0, o1["ones_r"]:o1["ones_r"] + H] = 1.0

    m1 = np.zeros((P, L1, G, CAP_D), np.float16)
    m1[info["m1_s"] % P, info["m1_l"], info["m1_s"] // P,
       info["m1_d"]] = info["m1_v"]
    blob2[:, o2["m1"]:o2["m1"] + L1 * G * CAP_D] = m1.reshape(P, -1)
    m3 = np.zeros((L3, CAP_D), np.float16)
    m3[info["m3_l"], info["m3_s"]] = info["m3_v"]
    blob2[0, o2["m3"]:o2["m3"] + L3 * CAP_D] = m3.reshape(-1)
    blob2[0:H, o2["w2"]:o2["w2"] + H] = np.asarray(W2, np.float16)
    blob2[0:H, o2["b1c"]] = np.asarray(b1, np.float16)
    return {"blob1": blob1, "blob2": blob2}


def build_phase1(nc, dims):
    G, capdeg, L1, L3 = dims
    o1, nb1 = _blob1_offsets(dims)
    o2, nb2 = _blob2_offsets(dims)

    b1_d = nc.dram_tensor("blob1", [P, nb1], F16, kind="ExternalInput")
    b2_d = nc.dram_tensor("blob2", [P, nb2], F16, kind="ExternalInput")
    india_d = nc.dram_tensor("india", [H, 1], F32, kind="ExternalOutput")

    with tile.TileContext(nc) as tc:
        with (
            tc.tile_pool(name="const", bufs=1) as const,
            tc.tile_pool(name="sm", bufs=8) as sm,
            tc.tile_pool(name="psa", bufs=3, space="PSUM") as psa,
            tc.tile_pool(name="psb", bufs=1, space="PSUM") as psb,
        ):
            b1t = const.tile([P, nb1], F16, tag="b1t")
            b2t = const.tile([P, nb2], F16, tag="b2t")
            nc.sync.dma_start(b1t[:], b1_d[:])
            nc.sync.dma_start(b2t[:], b2_d[:])

            ewd3 = b1t[:, o1["ewdeg"]:o1["ewdeg"] + G * capdeg].rearrange(
                "p (g c) -> p g c", c=capdeg)
            xTv = b1t[:, o1["xT"]:o1["xT"] + G * P]
            w1v = b1t[:, o1["w1"]:o1["w1"] + H]
            ewD = b1t[0:capdeg, o1["ewD"]:o1["ewD"] + capdeg]
            ones_c = b1t[0:capdeg, o1["ones_c"]:o1["ones_c"] + 1]
            ones_r = b1t[0:1, o1["ones_r"]:o1["ones_r"] + H]
            m1v = b2t[:, o2["m1"]:o2["m1"] + L1 * G * CAP_D]
            m3v = b2t[0:1, o2["m3"]:o2["m3"] + L3 * CAP_D]
            w2v = b2t[0:H, o2["w2"]:o2["w2"] + H]
            b1c = b2t[0:H, o2["b1c"]:o2["b1c"] + 1]

            # dis over all A nodes (partition layout) for the g1 scale
            deg = sm.tile([P, G], F32, tag="deg")
            dis = sm.tile([P, G], F32, tag="dis")
            nc.vector.reduce_sum(deg[:], ewd3, axis=AX.X)
            nc.scalar.activation(deg[:], deg[:], AF.Sqrt, bias=1.0)
            nc.vector.reciprocal(dis[:], deg[:])

            # dis over D as a row -> disX[h, d] = dis_d (rank-1 matmul)
            psdr = psb.tile([1, CAP_D], F32, tag="psdr")
            nc.tensor.matmul(psdr[:], ones_c, ewD[:, 0:CAP_D],
                             start=True, stop=True)
            sqr = sm.tile([1, CAP_D], F32, tag="sqr")
            nc.scalar.activation(sqr[:], psdr[:], AF.Sqrt, bias=1.0)
            dr = sm.tile([1, CAP_D], F32, tag="dr")
            nc.vector.reciprocal(dr[:], sqr[:])
            drh = sm.tile([1, CAP_D], F16, tag="drh")
            nc.scalar.copy(drh[:], dr[:])
            psdx = psb.tile([H, CAP_D], F32, tag="psdx")
            nc.tensor.matmul(psdx[:], ones_r, drh[:], start=True, stop=True)
            disX = sm.tile([H, CAP_D], F16, tag="disX")
            nc.scalar.copy(disX[:], psdx[:])

            # g1 = dis_A * (x[A] @ W1)
            g1 = const.tile([P, G * H], F16, tag="g1")
            for g in range(G):
                psg = psa.tile([P, H], F32, tag="psg")
                nc.tensor.matmul(psg[:], xTv[:, g * P:(g + 1) * P], w1v,
                                 start=True, stop=True)
                if g == 1:
                    nc.scalar.mul(g1[:, g * H:(g + 1) * H], psg[:],
                                  dis[:, g:g + 1])
                else:
                    nc.vector.tensor_scalar_mul(g1[:, g * H:(g + 1) * H],
                                                psg[:], dis[:, g:g + 1])

            # layer-2 row: dis0 * dis_d * M3[l, d], expanded over h by a
            # rank-1 matmul (all off the critical path)
            m3dr = sm.tile([1, L3 * CAP_D], F16, tag="m3dr")
            nc.vector.scalar_tensor_tensor(
                m3dr[:], m3v, dr[0:1, 0:1],
                drh[:].unsqueeze(1).broadcast_to((1, L3, CAP_D)),
                OP.mult, OP.mult)
            psm3 = psb.tile([H, L3 * CAP_D], F32, tag="psm3")
            nc.tensor.matmul(psm3[:], ones_r, m3dr[:], start=True, stop=True)
            m3sX = sm.tile([H, L3 * CAP_D], F16, tag="m3sX")
            nc.scalar.copy(m3sX[:], psm3[:])

            # layer-1 aggregation, transposed: ps1t[h, d] = sum_s g1[s,h]M1[s,d]
            ps1t = psb.tile([H, CAP_D], F32, tag="ps1t")
            k, nmm = 0, L1 * G
            for l in range(L1):
                for g in range(G):
                    nc.tensor.matmul(ps1t[:], g1[:, g * H:(g + 1) * H],
                                     m1v[:, (l * G + g) * CAP_D:
                                         (l * G + g + 1) * CAP_D],
                                     start=(k == 0), stop=(k == nmm - 1))
                    k += 1
            b1c32 = sm.tile([H, 1], F32, tag="b1c32")
            nc.scalar.copy(b1c32[:], b1c)
            u = sm.tile([H, CAP_D], F16, tag="u")
            nc.vector.tensor_tensor(u[:], ps1t[:], disX[:], op=OP.mult)
            h1T = sm.tile([H, CAP_D], F16, tag="h1T")
            nc.vector.tensor_scalar(h1T[:], u[:], b1c32[:], 0.0, OP.add, OP.max)

            # layer 2 collapsed: india[k] = sum_h W2[h,k] * sum_{l,d}
            #   h1T[h,d] * m3sX[h,(l,d)]   (+b2, relu applied in phase 2)
            u2 = sm.tile([H, L3 * CAP_D], F16, tag="u2")
            nc.vector.tensor_tensor(
                u2[:], h1T[:].unsqueeze(1).broadcast_to((H, L3, CAP_D)),
                m3sX[:].rearrange("p (l d) -> p l d", d=CAP_D), op=OP.mult)
            ps4 = psb.tile([H, L3 * CAP_D], F32, tag="ps4")
            nc.tensor.matmul(ps4[:], w2v, u2[:], start=True, stop=True)
            t2c = sm.tile([H, 1], F32, tag="t2c")
            nc.vector.reduce_sum(t2c[:], ps4[:], axis=AX.X)
            nc.sync.dma_start(india_d[:], t2c[:])
    nc.compile()
    return nc


def build_phase2(nc, t_steps, h):
    # column layout: wih|whh|hw|xaug|b2col
    owih, owhh, ohw, oxa = 0, 3 * h, 6 * h, 6 * h + 8
    ob2 = oxa + t_steps
    nbtot = ob2 + 2
    blob_d = nc.dram_tensor("blob", [h + 1, nbtot], F16, kind="ExternalInput")
    out_d = nc.dram_tensor("out", [8, 1], F32, kind="ExternalOutput")

    with tile.TileContext(nc) as tc:
        with (
            tc.tile_pool(name="const", bufs=1) as const,
            tc.tile_pool(name="sm", bufs=6) as sm,
            tc.tile_pool(name="psa", bufs=3, space="PSUM") as psa,
            tc.tile_pool(name="psb", bufs=1, space="PSUM") as psb,
        ):
            # dummy activation: hoists the (serial) activation-table load to
            # kernel start, off the gi critical path
            dum = sm.tile([1, 1], F32, tag="dum")
            nc.vector.memset(dum[:], 0.0)
            nc.scalar.activation(dum[:], dum[:], AF.Sigmoid)

            bt = const.tile([h + 1, nbtot], F16, tag="bt")
            nc.sync.dma_start(bt[:], blob_d[:])
            wih = bt[:, owih:owih + 3 * h]
            whh = bt[:, owhh:owhh + 3 * h]
            hw = bt[:, ohw:ohw + 8]
            xa = bt[:, oxa:oxa + t_steps]
            b2c32 = bt[:, ob2:ob2 + 2].bitcast(F32)

            haug = const.tile([h + 1, 1], F16, tag="haug")
            nc.vector.memset(haug[0:h, :], 0.0)
            nc.vector.memset(haug[h:h + 1, :], 1.0)

            # phase 1 emits raw pre-bias embeddings; apply +b2 and relu here
            # (the augmented ones-row has b2=0 and is relu-invariant; b2 is
            # packed as fp32 inside the fp16 blob and bitcast on read)
            xar = const.tile([h + 1, t_steps], F16, tag="xar")
            nc.vector.tensor_scalar(xar[:], xa, b2c32, 0.0, OP.add, OP.max)

            psg = psa.tile([h, 3 * t_steps], F32, tag="psg")
            for j in range(3):
                nc.tensor.matmul(psg[:, j * t_steps:(j + 1) * t_steps],
                                 wih[:, j * h:(j + 1) * h], xar[:],
                                 start=True, stop=True)
            gi_all = const.tile([h, 3 * t_steps], F16, tag="giall")
            nc.vector.tensor_copy(gi_all[:], psg[:])
            gir = gi_all[:, 0:t_steps]
            giz = gi_all[:, t_steps:2 * t_steps]
            gin = gi_all[:, 2 * t_steps:3 * t_steps]

            for t in range(t_steps):
                psr = psb.tile([h, 1], F32, tag="psr")
                nc.tensor.matmul(psr[:], whh[:, 0:h], haug[:],
                                 start=True, stop=True)
                psz = psb.tile([h, 1], F32, tag="psz")
                nc.tensor.matmul(psz[:], whh[:, h:2 * h], haug[:],
                                 start=True, stop=True)
                psn = psb.tile([h, 1], F32, tag="psn")
                nc.tensor.matmul(psn[:], whh[:, 2 * h:3 * h], haug[:],
                                 start=True, stop=True)
                r = sm.tile([h, 1], F32, tag="r")
                nc.scalar.activation(r[:], psr[:], AF.Sigmoid,
                                     bias=gir[:, t:t + 1])
                z = sm.tile([h, 1], F32, tag="z")
                nc.scalar.activation(z[:], psz[:], AF.Sigmoid,
                                     bias=giz[:, t:t + 1])
                n_t = sm.tile([h, 1], F16, tag="nt")
                nc.scalar.activation(n_t[:], psn[:], AF.Tanh,
                                     bias=gin[:, t:t + 1], scale=r[:])
                hm = sm.tile([h, 1], F16, tag="hm")
                nc.vector.tensor_sub(hm[:], haug[0:h, :], n_t[:])
                nc.vector.scalar_tensor_tensor(haug[0:h, :], hm[:], z[:],
                                               n_t[:], OP.mult, OP.add)

            ps_o = psb.tile([8, 1], F32, tag="pso")
            nc.tensor.matmul(ps_o[:], hw, haug[:], start=True, stop=True)
            o = sm.tile([8, 1], F32, tag="o")
            nc.scalar.activation(o[:], ps_o[:], AF.Sigmoid)
            nc.sync.dma_start(out_d[:], o[:])
    nc.compile()
    return nc


_P1_CACHE = {}
_P2_CACHE = {}

# Dev/profiling knobs (test.py pokes these; harness leaves defaults).
TRACE = False
LAST_RES = {}


def _get_phase1(dims):
    key = tuple(dims)
    if key not in _P1_CACHE:
        nc = bacc.Bacc("TRN2", target_bir_lowering=False, debug=False,
                       num_devices=T)
        _P1_CACHE[key] = build_phase1(nc, dims)
    return _P1_CACHE[key]


def _get_phase2():
    key = (T, H)
    if key not in _P2_CACHE:
        nc = bacc.Bacc("TRN2", target_bir_lowering=False, debug=False,
                       num_devices=1)
        _P2_CACHE[key] = build_phase2(nc, T, H)
    return _P2_CACHE[key]


def _p2_blob(seq, Wih, Whh, bih, bhh, headW, headb, b2):
    h, t_steps = H, T
    owih, owhh, ohw, oxa = 0, 3 * h, 6 * h, 6 * h + 8
    ob2 = oxa + t_steps
    blob = np.zeros((h + 1, ob2 + 2), np.float16)
    blob[0:h, ob2:ob2 + 2] = \
        np.asarray(b2, np.float32).view(np.float16).reshape(h, 2)
    blob[0:h, owih:owih + 3 * h] = np.asarray(Wih, np.float16).T
    blob[h, owih:owih + 3 * h] = np.asarray(bih, np.float16)
    blob[0:h, owhh:owhh + 3 * h] = np.asarray(Whh, np.float16).T
    blob[h, owhh:owhh + 3 * h] = np.asarray(bhh, np.float16)
    blob[0:h, ohw:ohw + 8] = np.asarray(headW, np.float16).T
    blob[h, ohw:ohw + 8] = np.asarray(headb, np.float16)
    blob[0:h, oxa:oxa + t_steps] = np.asarray(seq, np.float16).T
    blob[h, oxa:oxa + t_steps] = 1.0
    return blob


def kernel(x, edge_index, edge_weight, W1, b1, W2, b2, Wih, Whh, bih, bhh,
           headW, headb):
    x = np.asarray(x, np.float32)
    edge_index = np.asarray(edge_index)
    edge_weight = np.asarray(edge_weight, np.float32)

    infos = [_analyze(np.asarray(edge_index[t, 0]),
                      np.asarray(edge_index[t, 1]), edge_weight[t])
             for t in range(T)]
    dims = _dims_from(infos)
    nc1 = _get_phase1(dims)

    in_maps = [_fill_blobs(infos[t], x[t], W1, W2, b1, b2, dims)
               for t in range(T)]
    res1 = bass_utils.run_bass_kernel_spmd(nc1, in_maps,
                                           core_ids=list(range(T)),
                                           trace=TRACE)
    LAST_RES["p1"] = res1
    seq = np.stack([np.asarray(res1.results[t]["india"]).reshape(H)
                    for t in range(T)])

    nc2 = _get_phase2()
    in2 = [{"blob": _p2_blob(seq, Wih, Whh, bih, bhh, headW, headb, b2)}]
    res2 = bass_utils.run_bass_kernel_spmd(nc2, in2, core_ids=[0],
                                           trace=TRACE)
    LAST_RES["p2"] = res2
    return np.asarray(res2.results[0]["out"]).reshape(8).astype(np.float32)


# revision 14
# speedup vs baseline: 38.3356x; 1.0191x over previous
"""Trainium2 Bass kernel for nn_SanctionImpactGNN.

Temporal GNN: per timestep t (T=8) a 2-layer GCN over a 20000-node /
320000-edge graph; node-0 ("india") embeddings over time feed a tiny GRU +
sigmoid heads -> [8] output.

Key observation
---------------
The reference returns only h2[india] per graph.  That value depends solely on
node 0's 2-hop in-neighborhood:

  * D  = {0} u in-neighbors(0)           (~15-20 nodes)   -- layer-1 outputs
  * A  = D u in-neighbors(D)             (~250-350 nodes) -- layer-1 sources
  * layer-1 edges: all edges with dst in D (~300)
  * layer-2 edges: all edges with dst = 0 (~15-20)
  * degrees (for the symmetric GCN norm) of every node in A, which need the
    full in-edge weight lists of those nodes (~5000 edge weights).

Everything else in the graph is dead code w.r.t. the output, so the kernel
computes exactly this subgraph.  The host does *index* work only (masking,
packing, permutation, dtype packing); every floating-point operation stays on
device.

Per-core (one graph snapshot per NeuronCore, data-parallel over T):
  * deg_A = 1 + rowsum(ew slots)  -> dis_A = 1/sqrt(deg_A)   [partition axis]
  * deg_D via ones-matmul on a transposed slot pack -> dis as a row, expanded
    to disX[h,d] by a rank-1 matmul (keeps every dis scale off the critical
    path and avoids any PE transpose of activations)
  * g1 = dis_A * (x[A] @ W1)
  * h1^T = relu((g1^T-contracted M1 matmul) * disX + b1)   [M1 layered dense
    weighted adjacency; duplicate edges/self-loop collisions get their own
    layer so the host never sums weights]
  * g2 = h1 @ W2;  h2[0] = relu(dis_0 * (M3s^T @ g2) + b2)  [M3s rows are
    dis_D-scaled on device]
All matmuls run in fp16 (inputs quantized host-side; PSUM accumulates fp32).

Phase 2 (single core): 8-step GRU + sigmoid heads, biases folded via
augmented-ones rows, one fp16 blob load, gi for all steps precomputed, gate
math fused into Act ops (sigmoid/tanh with AP scale/bias).
"""

import numpy as np

import concourse.bacc as bacc
import concourse.mybir as mybir
import concourse.tile as tile
from concourse import bass_utils

F32 = mybir.dt.float32
F16 = mybir.dt.float16
AF = mybir.ActivationFunctionType
OP = mybir.AluOpType
AX = mybir.AxisListType

# Problem constants (hardcoded per contest contract).
T, N, E, F, H = 8, 20000, 320000, 128, 64
P = 128
INDIA = 0
CAP_D = 32  # max |{0} u in-neighbors(0)| supported (observed ~20)


def _analyze(src, dst, w):
    """Pure-index extraction of node 0's 2-hop in-neighborhood."""
    m0 = dst == INDIA
    s1 = np.unique(src[m0])
    D = np.concatenate([[INDIA], s1[s1 != INDIA]]).astype(np.int64)
    nD = len(D)
    assert nD <= CAP_D, f"|D|={nD} exceeds CAP_D={CAP_D}"
    mDe = np.isin(dst, D)
    extra = np.setdiff1d(np.unique(src[mDe]), D)
    A = np.concatenate([D, extra.astype(np.int64)])
    nA = len(A)
    pos = np.full(N, -1, np.int64)
    pos[A] = np.arange(nA)

    # per-A-node in-edge weight slots (partition-axis degree layout)
    mA = np.isin(dst, A)
    rdeg = pos[dst[mA]]
    o = np.argsort(rdeg, kind="stable")
    rdeg = rdeg[o]
    kdeg = np.arange(len(rdeg)) - np.searchsorted(rdeg, rdeg, "left")
    vdeg = np.asarray(w[mA], np.float32)[o]

    # layer-1 weighted adjacency entries (incl. unit self-loops), layered so
    # coincident (src,dst) cells never need host-side summation
    sM = np.concatenate([pos[src[mDe]], np.arange(nD)])
    dM = np.concatenate([pos[dst[mDe]], np.arange(nD)])
    vM = np.concatenate([np.asarray(w[mDe], np.float32),
                         np.ones(nD, np.float32)])
    assert (sM >= 0).all() and (dM >= 0).all() and (dM < nD).all()
    key = sM * CAP_D + dM
    o1 = np.argsort(key, kind="stable")
    ks = key[o1]
    lM = np.arange(len(ks)) - np.searchsorted(ks, ks, "left")

    # layer-2 entries: edges into node 0 (+ its self-loop)
    s3 = np.concatenate([pos[src[m0]], [0]])
    v3 = np.concatenate([np.asarray(w[m0], np.float32),
                         np.ones(1, np.float32)])
    assert (s3 >= 0).all() and (s3 < nD).all()
    o3 = np.argsort(s3, kind="stable")
    s3 = s3[o3]
    l3 = np.arange(len(s3)) - np.searchsorted(s3, s3, "left")

    return dict(A=A, nA=nA, nD=nD,
                deg_r=rdeg, deg_k=kdeg, deg_v=vdeg,
                m1_s=sM[o1], m1_d=dM[o1], m1_l=lM, m1_v=vM[o1],
                m3_s=s3, m3_l=l3, m3_v=v3[o3])


def _dims_from(infos):
    G = max(1, -(-max(i["nA"] for i in infos) // P))
    capdeg = max(8, int(max(i["deg_k"].max() + 1 if len(i["deg_k"]) else 1
                            for i in infos)))
    capdeg = (capdeg + 3) // 4 * 4
    L1 = int(max(i["m1_l"].max() + 1 for i in infos))
    L3 = int(max(i["m3_l"].max() + 1 for i in infos))
    return G, capdeg, L1, L3


def _blob1_offsets(dims):
    G, capdeg, L1, L3 = dims
    o = {}
    c = 0
    for name, width in (("ewdeg", G * capdeg), ("xT", G * P), ("w1", H),
                        ("ewD", max(capdeg, CAP_D)), ("ones_c", 1), ("ones_r", H)):
        o[name] = c
        c += width
    return o, c


def _blob2_offsets(dims):
    G, capdeg, L1, L3 = dims
    o = {}
    c = 0
    for name, width in (("m1", L1 * G * CAP_D), ("m3", L3 * CAP_D),
                        ("w2", H), ("b1r", H)):
        o[name] = c
        c += width
    return o, c


def _fill_blobs(info, x_t, W1, W2, b1, b2, dims):
    G, capdeg, L1, L3 = dims
    nA, nD = info["nA"], info["nD"]
    o1, nb1 = _blob1_offsets(dims)
    o2, nb2 = _blob2_offsets(dims)
    blob1 = np.zeros((P, nb1), np.float16)
    blob2 = np.zeros((P, nb2), np.float16)

    ewdeg = np.zeros((P, G, capdeg), np.float16)
    ewdeg[info["deg_r"] % P, info["deg_r"] // P, info["deg_k"]] = info["deg_v"]
    blob1[:, o1["ewdeg"]:o1["ewdeg"] + G * capdeg] = ewdeg.reshape(P, -1)
    blob1[:, o1["xT"]:o1["xT"] + nA] = \
        np.asarray(x_t, np.float32)[info["A"]].T.astype(np.float16)
    blob1[:, o1["w1"]:o1["w1"] + H] = np.asarray(W1, np.float16)
    # transposed in-edge slots for D nodes only (slot on partition, node on
    # free) -> degrees of D as a row via ones-matmul
    mD = info["deg_r"] < nD
    blob1[info["deg_k"][mD], o1["ewD"] + info["deg_r"][mD]] = \
        info["deg_v"][mD].astype(np.float16)
    blob1[0:capdeg, o1["ones_c"]] = 1.0
    blob1[0, o1["ones_r"]:o1["ones_r"] + H] = 1.0

    m1 = np.zeros((P, L1, G, CAP_D), np.float16)
    m1[info["m1_s"] % P, info["m1_l"], info["m1_s"] // P,
       info["m1_d"]] = info["m1_v"]
    blob2[:, o2["m1"]:o2["m1"] + L1 * G * CAP_D] = m1.reshape(P, -1)
    m3 = np.zeros((L3, CAP_D), np.float16)
    m3[info["m3_l"], info["m3_s"]] = info["m3_v"]
    blob2[0, o2["m3"]:o2["m3"] + L3 * CAP_D] = m3.reshape(-1)
    blob2[0:H, o2["w2"]:o2["w2"] + H] = np.asarray(W2, np.float16)
    blob2[0, o2["b1r"]:o2["b1r"] + H] = np.asarray(b1, np.float16)
    return {"blob1": blob1, "blob2": blob2}


def build_phase1(nc, dims):
    G, capdeg, L1, L3 = dims
    o1, nb1 = _blob1_offsets(dims)
    o2, nb2 = _blob2_offsets(dims)

    b1_d = nc.dram_tensor("blob1", [P, nb1], F16, kind="ExternalInput")
    b2_d = nc.dram_tensor("blob2", [P, nb2], F16, kind="ExternalInput")
    india_d = nc.dram_tensor("india", [H, 1], F32, kind="ExternalOutput")

    with tile.TileContext(nc) as tc:
        with (
            tc.tile_pool(name="const", bufs=1) as const,
            tc.tile_pool(name="sm", bufs=8) as sm,
            tc.tile_pool(name="psa", bufs=3, space="PSUM") as psa,
            tc.tile_pool(name="psb", bufs=1, space="PSUM") as psb,
        ):
            b1t = const.tile([P, nb1], F16, tag="b1t")
            b2t = const.tile([P, nb2], F16, tag="b2t")
            nc.sync.dma_start(b1t[:], b1_d[:])
            nc.sync.dma_start(b2t[:], b2_d[:])

            ewd3 = b1t[:, o1["ewdeg"]:o1["ewdeg"] + G * capdeg].rearrange(
                "p (g c) -> p g c", c=capdeg)
            xTv = b1t[:, o1["xT"]:o1["xT"] + G * P]
            w1v = b1t[:, o1["w1"]:o1["w1"] + H]
            ewD = b1t[0:capdeg, o1["ewD"]:o1["ewD"] + capdeg]
            ones_c = b1t[0:capdeg, o1["ones_c"]:o1["ones_c"] + 1]
            ones_r = b1t[0:1, o1["ones_r"]:o1["ones_r"] + H]
            m1v = b2t[:, o2["m1"]:o2["m1"] + L1 * G * CAP_D]
            m3v = b2t[0:1, o2["m3"]:o2["m3"] + L3 * CAP_D]
            w2v = b2t[0:H, o2["w2"]:o2["w2"] + H]
            b1r = b2t[0:1, o2["b1r"]:o2["b1r"] + H]

            # dis over all A nodes (partition layout) for the g1 scale
            deg = sm.tile([P, G], F32, tag="deg")
            dis = sm.tile([P, G], F32, tag="dis")
            nc.vector.reduce_sum(deg[:], ewd3, axis=AX.X)
            nc.scalar.activation(deg[:], deg[:], AF.Sqrt, bias=1.0)
            nc.vector.reciprocal(dis[:], deg[:])

            # dis over D as a row -> disX[h, d] = dis_d (rank-1 matmul)
            psdr = psb.tile([1, CAP_D], F32, tag="psdr")
            nc.tensor.matmul(psdr[:], ones_c, ewD[:, 0:CAP_D],
                             start=True, stop=True)
            sqr = sm.tile([1, CAP_D], F32, tag="sqr")
            nc.scalar.activation(sqr[:], psdr[:], AF.Sqrt, bias=1.0)
            dr = sm.tile([1, CAP_D], F32, tag="dr")
            nc.vector.reciprocal(dr[:], sqr[:])
            sqrh = sm.tile([1, CAP_D], F16, tag="sqrh")
            nc.scalar.copy(sqrh[:], sqr[:])

            # g1 = dis_A * (x[A] @ W1)
            g1 = const.tile([P, G * H], F16, tag="g1")
            for g in range(G):
                psg = psa.tile([P, H], F32, tag="psg")
                nc.tensor.matmul(psg[:], xTv[:, g * P:(g + 1) * P], w1v,
                                 start=True, stop=True)
                if g == 2:
                    nc.scalar.mul(g1[:, g * H:(g + 1) * H], psg[:],
                                  dis[:, g:g + 1])
                else:
                    nc.vector.tensor_scalar_mul(g1[:, g * H:(g + 1) * H],
                                                psg[:], dis[:, g:g + 1])

            # layer-2 row: QX[l,d] = dis0 * dis_d^2 * M3[l,d], expanded over
            # h by a rank-1 matmul (Pool engine; off the critical path)
            drsq = sm.tile([1, CAP_D], F16, tag="drsq")
            nc.gpsimd.tensor_tensor(drsq[:], dr[:], dr[:], op=OP.mult)
            m3dr = sm.tile([1, L3 * CAP_D], F16, tag="m3dr")
            nc.vector.scalar_tensor_tensor(
                m3dr[:], m3v, dr[0:1, 0:1],
                drsq[:].unsqueeze(1).broadcast_to((1, L3, CAP_D)),
                OP.mult, OP.mult)
            psm3 = psb.tile([H, L3 * CAP_D], F32, tag="psm3")
            nc.tensor.matmul(psm3[:], ones_r, m3dr[:], start=True, stop=True)
            qx = sm.tile([H, L3 * CAP_D], F16, tag="qx")
            nc.vector.tensor_copy(qx[:], psm3[:])

            # layer-1 aggregation, transposed: ps1t[h, d] = sum_s g1[s,h]M1[s,d]
            # ps1t[h,d] = sum_s g1[s,h] M1[s,d] + b1[h] sqrt(deg_d+1); with
            # that bias row folded in, relu commutes past the positive scales:
            # u2 = max(ps1t, 0) * QX in a single fused DVE op
            ps1t = psb.tile([H, CAP_D], F32, tag="ps1t")
            k, nmm = 0, L1 * G + 1
            for l in range(L1):
                for g in range(G):
                    nc.tensor.matmul(ps1t[:], g1[:, g * H:(g + 1) * H],
                                     m1v[:, (l * G + g) * CAP_D:
                                         (l * G + g + 1) * CAP_D],
                                     start=(k == 0), stop=False)
                    k += 1
            nc.tensor.matmul(ps1t[:], b1r, sqrh[:], start=False, stop=True)

            # layer 2 collapsed: india[k] = sum_h W2[h,k] * sum_{l,d}
            #   max(ps1t[h,d],0) * QX[h,(l,d)]  (+b2, relu applied in phase 2)
            u2 = sm.tile([H, L3 * CAP_D], F16, tag="u2")
            nc.vector.scalar_tensor_tensor(
                u2[:], ps1t[:].unsqueeze(1).broadcast_to((H, L3, CAP_D)), 0.0,
                qx[:].rearrange("p (l d) -> p l d", d=CAP_D), OP.max, OP.mult)
            ps4 = psb.tile([H, L3 * CAP_D], F32, tag="ps4")
            nc.tensor.matmul(ps4[:], w2v, u2[:], start=True, stop=True)
            t2c = sm.tile([H, 1], F32, tag="t2c")
            nc.vector.reduce_sum(t2c[:], ps4[:], axis=AX.X)
            nc.sync.dma_start(india_d[:], t2c[:])
    nc.compile()
    return nc


def build_phase2(nc, t_steps, h):
    # column layout: wih|whh|hw|xaug|b2col
    owih, owhh, ohw, oxa = 0, 3 * h, 6 * h, 6 * h + 8
    ob2 = oxa + t_steps
    nbtot = ob2 + 2
    blob_d = nc.dram_tensor("blob", [h + 1, nbtot], F16, kind="ExternalInput")
    out_d = nc.dram_tensor("out", [8, 1], F32, kind="ExternalOutput")

    with tile.TileContext(nc) as tc:
        with (
            tc.tile_pool(name="const", bufs=1) as const,
            tc.tile_pool(name="sm", bufs=6) as sm,
            tc.tile_pool(name="psa", bufs=3, space="PSUM") as psa,
            tc.tile_pool(name="psb", bufs=1, space="PSUM") as psb,
        ):
            # dummy activation: hoists the (serial) activation-table load to
            # kernel start, off the gi critical path
            dum = sm.tile([1, 1], F32, tag="dum")
            nc.vector.memset(dum[:], 0.0)
            nc.scalar.activation(dum[:], dum[:], AF.Sigmoid)

            bt = const.tile([h + 1, nbtot], F16, tag="bt")
            nc.sync.dma_start(bt[:], blob_d[:])
            wih = bt[:, owih:owih + 3 * h]
            whh = bt[:, owhh:owhh + 3 * h]
            hw = bt[:, ohw:ohw + 8]
            xa = bt[:, oxa:oxa + t_steps]
            b2c32 = bt[:, ob2:ob2 + 2].bitcast(F32)

            haug = const.tile([h + 1, 1], F16, tag="haug")
            nc.vector.memset(haug[0:h, :], 0.0)
            nc.vector.memset(haug[h:h + 1, :], 1.0)

            # phase 1 emits raw pre-bias embeddings; apply +b2 and relu here
            # (the augmented ones-row has b2=0 and is relu-invariant; b2 is
            # packed as fp32 inside the fp16 blob and bitcast on read)
            xar = const.tile([h + 1, t_steps], F16, tag="xar")
            nc.vector.tensor_scalar(xar[:], xa, b2c32, 0.0, OP.add, OP.max)

            psg = psa.tile([h, 3 * t_steps], F32, tag="psg")
            for j in range(3):
                nc.tensor.matmul(psg[:, j * t_steps:(j + 1) * t_steps],
                                 wih[:, j * h:(j + 1) * h], xar[:],
                                 start=True, stop=True)
            gi_all = const.tile([h, 3 * t_steps], F16, tag="giall")
            nc.vector.tensor_copy(gi_all[:], psg[:])
            gir = gi_all[:, 0:t_steps]
            giz = gi_all[:, t_steps:2 * t_steps]
            gin = gi_all[:, 2 * t_steps:3 * t_steps]

            for t in range(t_steps):
                psr = psb.tile([h, 1], F32, tag="psr")
                nc.tensor.matmul(psr[:], whh[:, 0:h], haug[:],
                                 start=True, stop=True)
                psz = psb.tile([h, 1], F32, tag="psz")
                nc.tensor.matmul(psz[:], whh[:, h:2 * h], haug[:],
                                 start=True, stop=True)
                psn = psb.tile([h, 1], F32, tag="psn")
                nc.tensor.matmul(psn[:], whh[:, 2 * h:3 * h], haug[:],
                                 start=True, stop=True)
                r = sm.tile([h, 1], F32, tag="r")
                nc.scalar.activation(r[:], psr[:], AF.Sigmoid,
                                     bias=gir[:, t:t + 1])
                z = sm.tile([h, 1], F32, tag="z")
                nc.scalar.activation(z[:], psz[:], AF.Sigmoid,
                                     bias=giz[:, t:t + 1])
                n_t = sm.tile([h, 1], F16, tag="nt")
                nc.scalar.activation(n_t[:], psn[:], AF.Tanh,
                                     bias=gin[:, t:t + 1], scale=r[:])
                hm = sm.tile([h, 1], F16, tag="hm")
                nc.vector.tensor_sub(hm[:], haug[0:h, :], n_t[:])
                nc.vector.scalar_tensor_tensor(haug[0:h, :], hm[:], z[:],
                                               n_t[:], OP.mult, OP.add)

            ps_o = psb.tile([8, 1], F32, tag="pso")
            nc.tensor.matmul(ps_o[:], hw, haug[:], start=True, stop=True)
            o = sm.tile([8, 1], F32, tag="o")
            nc.scalar.activation(o[:], ps_o[:], AF.Sigmoid)
            nc.sync.dma_start(out_d[:], o[:])
    nc.compile()
    return nc


_P1_CACHE = {}
_P2_CACHE = {}

# Dev/profiling knobs (test.py pokes these; harness leaves defaults).
TRACE = False
LAST_RES = {}


def _get_phase1(dims):
    key = tuple(dims)
    if key not in _P1_CACHE:
        nc = bacc.Bacc("TRN2", target_bir_lowering=False, debug=False,
                       num_devices=T)
        _P1_CACHE[key] = build_phase1(nc, dims)
    return _P1_CACHE[key]


def _get_phase2():
    key = (T, H)
    if key not in _P2_CACHE:
        nc = bacc.Bacc("TRN2", target_bir_lowering=False, debug=False,
                       num_devices=1)
        _P2_CACHE[key] = build_phase2(nc, T, H)
    return _P2_CACHE[key]


def _p2_blob(seq, Wih, Whh, bih, bhh, headW, headb, b2):
    h, t_steps = H, T
    owih, owhh, ohw, oxa = 0, 3 * h, 6 * h, 6 * h + 8
    ob2 = oxa + t_steps
    blob = np.zeros((h + 1, ob2 + 2), np.float16)
    blob[0:h, ob2:ob2 + 2] = \
        np.asarray(b2, np.float32).view(np.float16).reshape(h, 2)
    blob[0:h, owih:owih + 3 * h] = np.asarray(Wih, np.float16).T
    blob[h, owih:owih + 3 * h] = np.asarray(bih, np.float16)
    blob[0:h, owhh:owhh + 3 * h] = np.asarray(Whh, np.float16).T
    blob[h, owhh:owhh + 3 * h] = np.asarray(bhh, np.float16)
    blob[0:h, ohw:ohw + 8] = np.asarray(headW, np.float16).T
    blob[h, ohw:ohw + 8] = np.asarray(headb, np.float16)
    blob[0:h, oxa:oxa + t_steps] = np.asarray(seq, np.float16).T
    blob[h, oxa:oxa + t_steps] = 1.0
    return blob


def kernel(x, edge_index, edge_weight, W1, b1, W2, b2, Wih, Whh, bih, bhh,
           headW, headb):
    x = np.asarray(x, np.float32)
    edge_index = np.asarray(edge_index)
    edge_weight = np.asarray(edge_weight, np.float32)

    infos = [_analyze(np.asarray(edge_index[t, 0]),
                      np.asarray(edge_index[t, 1]), edge_weight[t])
             for t in range(T)]
    dims = _dims_from(infos)
    nc1 = _get_phase1(dims)

    in_maps = [_fill_blobs(infos[t], x[t], W1, W2, b1, b2, dims)
               for t in range(T)]
    res1 = bass_utils.run_bass_kernel_spmd(nc1, in_maps,
                                           core_ids=list(range(T)),
                                           trace=TRACE)
    LAST_RES["p1"] = res1
    seq = np.stack([np.asarray(res1.results[t]["india"]).reshape(H)
                    for t in range(T)])

    nc2 = _get_phase2()
    in2 = [{"blob": _p2_blob(seq, Wih, Whh, bih, bhh, headW, headb, b2)}]
    res2 = bass_utils.run_bass_kernel_spmd(nc2, in2, core_ids=[0],
                                           trace=TRACE)
    LAST_RES["p2"] = res2
    return np.asarray(res2.results[0]["out"]).reshape(8).astype(np.float32)


# revision 25
# speedup vs baseline: 38.4060x; 1.0018x over previous
"""Trainium2 Bass kernel for nn_SanctionImpactGNN.

Temporal GNN: per timestep t (T=8) a 2-layer GCN over a 20000-node /
320000-edge graph; node-0 ("india") embeddings over time feed a tiny GRU +
sigmoid heads -> [8] output.

Key observation
---------------
The reference returns only h2[india] per graph.  That value depends solely on
node 0's 2-hop in-neighborhood:

  * D  = {0} u in-neighbors(0)           (~15-20 nodes)   -- layer-1 outputs
  * A  = D u in-neighbors(D)             (~250-350 nodes) -- layer-1 sources
  * layer-1 edges: all edges with dst in D (~300)
  * layer-2 edges: all edges with dst = 0 (~15-20)
  * degrees (for the symmetric GCN norm) of every node in A, which need the
    full in-edge weight lists of those nodes (~5000 edge weights).

Everything else in the graph is dead code w.r.t. the output, so the kernel
computes exactly this subgraph.  The host does *index* work only (masking,
packing, permutation, dtype packing); every floating-point operation stays on
device.

Per-core (one graph snapshot per NeuronCore, data-parallel over T):
  * deg_A = 1 + rowsum(ew slots)  -> dis_A = 1/sqrt(deg_A)   [partition axis]
  * deg_D via ones-matmul on a transposed slot pack -> dis as a row, expanded
    to disX[h,d] by a rank-1 matmul (keeps every dis scale off the critical
    path and avoids any PE transpose of activations)
  * g1 = dis_A * (x[A] @ W1)
  * h1^T = relu((g1^T-contracted M1 matmul) * disX + b1)   [M1 layered dense
    weighted adjacency; duplicate edges/self-loop collisions get their own
    layer so the host never sums weights]
  * g2 = h1 @ W2;  h2[0] = relu(dis_0 * (M3s^T @ g2) + b2)  [M3s rows are
    dis_D-scaled on device]
All matmuls run in fp16 (inputs quantized host-side; PSUM accumulates fp32).

Phase 2 (single core): 8-step GRU + sigmoid heads, biases folded via
augmented-ones rows, one fp16 blob load, gi for all steps precomputed, gate
math fused into Act ops (sigmoid/tanh with AP scale/bias).
"""

import numpy as np

import concourse.bacc as bacc
import concourse.mybir as mybir
import concourse.tile as tile
from concourse import bass_utils

F32 = mybir.dt.float32
F16 = mybir.dt.float16
AF = mybir.ActivationFunctionType
OP = mybir.AluOpType
AX = mybir.AxisListType

# Problem constants (hardcoded per contest contract).
T, N, E, F, H = 8, 20000, 320000, 128, 64
P = 128
INDIA = 0
CAP_D = 32  # max |{0} u in-neighbors(0)| supported (observed ~20)


def _analyze(src, dst, w):
    """Pure-index extraction of node 0's 2-hop in-neighborhood."""
    m0 = dst == INDIA
    s1 = np.unique(src[m0])
    D = np.concatenate([[INDIA], s1[s1 != INDIA]]).astype(np.int64)
    nD = len(D)
    assert nD <= CAP_D, f"|D|={nD} exceeds CAP_D={CAP_D}"
    mDe = np.isin(dst, D)
    extra = np.setdiff1d(np.unique(src[mDe]), D)
    A = np.concatenate([D, extra.astype(np.int64)])
    nA = len(A)
    pos = np.full(N, -1, np.int64)
    pos[A] = np.arange(nA)

    # per-A-node in-edge weight slots (partition-axis degree layout)
    mA = np.isin(dst, A)
    rdeg = pos[dst[mA]]
    o = np.argsort(rdeg, kind="stable")
    rdeg = rdeg[o]
    kdeg = np.arange(len(rdeg)) - np.searchsorted(rdeg, rdeg, "left")
    vdeg = np.asarray(w[mA], np.float32)[o]

    # layer-1 weighted adjacency entries (incl. unit self-loops), layered so
    # coincident (src,dst) cells never need host-side summation
    sM = np.concatenate([pos[src[mDe]], np.arange(nD)])
    dM = np.concatenate([pos[dst[mDe]], np.arange(nD)])
    vM = np.concatenate([np.asarray(w[mDe], np.float32),
                         np.ones(nD, np.float32)])
    assert (sM >= 0).all() and (dM >= 0).all() and (dM < nD).all()
    key = sM * CAP_D + dM
    o1 = np.argsort(key, kind="stable")
    ks = key[o1]
    lM = np.arange(len(ks)) - np.searchsorted(ks, ks, "left")

    # layer-2 entries: edges into node 0 (+ its self-loop)
    s3 = np.concatenate([pos[src[m0]], [0]])
    v3 = np.concatenate([np.asarray(w[m0], np.float32),
                         np.ones(1, np.float32)])
    assert (s3 >= 0).all() and (s3 < nD).all()
    o3 = np.argsort(s3, kind="stable")
    s3 = s3[o3]
    l3 = np.arange(len(s3)) - np.searchsorted(s3, s3, "left")

    return dict(A=A, nA=nA, nD=nD,
                deg_r=rdeg, deg_k=kdeg, deg_v=vdeg,
                m1_s=sM[o1], m1_d=dM[o1], m1_l=lM, m1_v=vM[o1],
                m3_s=s3, m3_l=l3, m3_v=v3[o3])


def _dims_from(infos):
    G = max(1, -(-max(i["nA"] for i in infos) // P))
    capdeg = max(8, int(max(i["deg_k"].max() + 1 if len(i["deg_k"]) else 1
                            for i in infos)))
    capdeg = (capdeg + 3) // 4 * 4
    L1 = int(max(i["m1_l"].max() + 1 for i in infos))
    L3 = int(max(i["m3_l"].max() + 1 for i in infos))
    return G, capdeg, L1, L3


def _blob1_offsets(dims):
    G, capdeg, L1, L3 = dims
    o = {}
    c = 0
    for name, width in (("ewdeg", G * capdeg), ("w1", H),
                        ("ewD", max(capdeg, CAP_D)), ("ones_c", 1),
                        ("ones_r", H), ("m1", L1 * G * CAP_D),
                        ("m3", L3 * CAP_D), ("w2", H), ("b1r", H)):
        o[name] = c
        c += width
    return o, c


def _fill_blobs(info, x_t, W1, W2, b1, b2, dims):
    G, capdeg, L1, L3 = dims
    nA, nD = info["nA"], info["nD"]
    o1, nb1 = _blob1_offsets(dims)
    blob1 = np.zeros((P, nb1), np.float16)
    blob2 = np.zeros((P, G * P), np.float16)
    blob2[:, 0:nA] = np.asarray(x_t, np.float32)[info["A"]].T.astype(np.float16)

    ewdeg = np.zeros((P, G, capdeg), np.float16)
    ewdeg[info["deg_r"] % P, info["deg_r"] // P, info["deg_k"]] = info["deg_v"]
    blob1[:, o1["ewdeg"]:o1["ewdeg"] + G * capdeg] = ewdeg.reshape(P, -1)
    blob1[:, o1["w1"]:o1["w1"] + H] = np.asarray(W1, np.float16)
    # transposed in-edge slots for D nodes only (slot on partition, node on
    # free) -> degrees of D as a row via ones-matmul
    mD = info["deg_r"] < nD
    blob1[info["deg_k"][mD], o1["ewD"] + info["deg_r"][mD]] = \
        info["deg_v"][mD].astype(np.float16)
    blob1[0:capdeg, o1["ones_c"]] = 1.0
    blob1[0, o1["ones_r"]:o1["ones_r"] + H] = 1.0

    m1 = np.zeros((P, L1, G, CAP_D), np.float16)
    m1[info["m1_s"] % P, info["m1_l"], info["m1_s"] // P,
       info["m1_d"]] = info["m1_v"]
    blob1[:, o1["m1"]:o1["m1"] + L1 * G * CAP_D] = m1.reshape(P, -1)
    m3 = np.zeros((L3, CAP_D), np.float16)
    m3[info["m3_l"], info["m3_s"]] = info["m3_v"]
    blob1[0, o1["m3"]:o1["m3"] + L3 * CAP_D] = m3.reshape(-1)
    blob1[0:H, o1["w2"]:o1["w2"] + H] = np.asarray(W2, np.float16)
    blob1[0, o1["b1r"]:o1["b1r"] + H] = np.asarray(b1, np.float16)
    return {"blob1": blob1, "blob2": blob2}


def build_phase1(nc, dims):
    G, capdeg, L1, L3 = dims
    o1, nb1 = _blob1_offsets(dims)

    b1_d = nc.dram_tensor("blob1", [P, nb1], F16, kind="ExternalInput")
    b2_d = nc.dram_tensor("blob2", [P, G * P], F16, kind="ExternalInput")
    india_d = nc.dram_tensor("india", [H, 1], F32, kind="ExternalOutput")

    with tile.TileContext(nc) as tc:
        with (
            tc.tile_pool(name="const", bufs=1) as const,
            tc.tile_pool(name="sm", bufs=8) as sm,
            tc.tile_pool(name="psa", bufs=3, space="PSUM") as psa,
            tc.tile_pool(name="psb", bufs=1, space="PSUM") as psb,
        ):
            b1t = const.tile([P, nb1], F16, tag="b1t")
            b2t = const.tile([P, G * P], F16, tag="b2t")
            nc.sync.dma_start(b1t[:], b1_d[:])
            nc.sync.dma_start(b2t[:], b2_d[:])

            ewd3 = b1t[:, o1["ewdeg"]:o1["ewdeg"] + G * capdeg].rearrange(
                "p (g c) -> p g c", c=capdeg)
            xTv = b2t[:, :]
            w1v = b1t[:, o1["w1"]:o1["w1"] + H]
            ewD = b1t[0:capdeg, o1["ewD"]:o1["ewD"] + capdeg]
            ones_c = b1t[0:capdeg, o1["ones_c"]:o1["ones_c"] + 1]
            ones_r = b1t[0:1, o1["ones_r"]:o1["ones_r"] + H]
            m1v = b1t[:, o1["m1"]:o1["m1"] + L1 * G * CAP_D]
            m3v = b1t[0:1, o1["m3"]:o1["m3"] + L3 * CAP_D]
            w2v = b1t[0:H, o1["w2"]:o1["w2"] + H]
            b1r = b1t[0:1, o1["b1r"]:o1["b1r"] + H]

            # dis over all A nodes (partition layout) for the g1 scale
            deg = sm.tile([P, G], F32, tag="deg")
            dis = sm.tile([P, G], F32, tag="dis")
            nc.vector.reduce_sum(deg[:], ewd3, axis=AX.X)
            nc.scalar.activation(deg[:], deg[:], AF.Sqrt, bias=1.0)
            nc.vector.reciprocal(dis[:], deg[:])

            # dis over D as a row -> disX[h, d] = dis_d (rank-1 matmul)
            psdr = psb.tile([1, CAP_D], F32, tag="psdr")
            nc.tensor.matmul(psdr[:], ones_c, ewD[:, 0:CAP_D],
                             start=True, stop=True)
            sqr = sm.tile([1, CAP_D], F32, tag="sqr")
            nc.scalar.activation(sqr[:], psdr[:], AF.Sqrt, bias=1.0)
            dr = sm.tile([1, CAP_D], F32, tag="dr")
            nc.vector.reciprocal(dr[:], sqr[:])

            # g1 = dis_A * (x[A] @ W1)
            g1 = const.tile([P, G * H], F16, tag="g1")
            for g in range(G):
                psg = psa.tile([P, H], F32, tag="psg")
                nc.tensor.matmul(psg[:], xTv[:, g * P:(g + 1) * P], w1v,
                                 start=True, stop=True)
                if g == 2:
                    nc.scalar.mul(g1[:, g * H:(g + 1) * H], psg[:],
                                  dis[:, g:g + 1])
                else:
                    nc.vector.tensor_scalar_mul(g1[:, g * H:(g + 1) * H],
                                                psg[:], dis[:, g:g + 1])


            # layer-1 aggregation, transposed: ps1t[h, d] = sum_s g1[s,h]M1[s,d]
            # (off critical path) layer-2 row: QX[l,d] = dis0 * dis_d^2 * M3[l,d], expanded over
            # h by a rank-1 matmul (Pool engine; off the critical path)
            drsq = sm.tile([1, CAP_D], F16, tag="drsq")
            nc.gpsimd.tensor_tensor(drsq[:], dr[:], dr[:], op=OP.mult)
            m3dr = sm.tile([1, L3 * CAP_D], F16, tag="m3dr")
            nc.vector.scalar_tensor_tensor(
                m3dr[:], m3v, dr[0:1, 0:1],
                drsq[:].unsqueeze(1).broadcast_to((1, L3, CAP_D)),
                OP.mult, OP.mult)
            psm3 = psb.tile([H, L3 * CAP_D], F32, tag="psm3")
            nc.tensor.matmul(psm3[:], ones_r, m3dr[:], start=True, stop=True)
            qx = sm.tile([H, L3 * CAP_D], F16, tag="qx")
            nc.vector.tensor_copy(qx[:], psm3[:])
            sqrh = sm.tile([1, CAP_D], F16, tag="sqrh")
            nc.scalar.copy(sqrh[:], sqr[:])

            # ps1t[h,d] = sum_s g1[s,h] M1[s,d] + b1[h] sqrt(deg_d+1); with
            # that bias row folded in, relu commutes past the positive scales:
            # u2 = max(ps1t, 0) * QX in a single fused DVE op
            ps1t = psb.tile([H, CAP_D], F32, tag="ps1t")
            nc.tensor.matmul(ps1t[:], b1r, sqrh[:], start=True, stop=False)
            k, nmm = 0, L1 * G
            for l in range(L1):
                for g in range(G):
                    nc.tensor.matmul(ps1t[:], g1[:, g * H:(g + 1) * H],
                                     m1v[:, (l * G + g) * CAP_D:
                                         (l * G + g + 1) * CAP_D],
                                     start=False, stop=(k == nmm - 1))
                    k += 1

            # layer 2 collapsed: india[k] = sum_h W2[h,k] * sum_{l,d}
            #   max(ps1t[h,d],0) * QX[h,(l,d)]  (+b2, relu applied in phase 2)
            u2 = sm.tile([H, L3 * CAP_D], F16, tag="u2")
            nc.vector.scalar_tensor_tensor(
                u2[:], ps1t[:].unsqueeze(1).broadcast_to((H, L3, CAP_D)), 0.0,
                qx[:].rearrange("p (l d) -> p l d", d=CAP_D), OP.max, OP.mult)
            ps4 = psb.tile([H, L3 * CAP_D], F32, tag="ps4")
            nc.tensor.matmul(ps4[:], w2v, u2[:], start=True, stop=True)
            t2c = sm.tile([H, 1], F32, tag="t2c")
            nc.vector.reduce_sum(t2c[:], ps4[:], axis=AX.X)
            nc.sync.dma_start(india_d[:], t2c[:])
    nc.compile()
    return nc


def build_phase2(nc, t_steps, h):
    # column layout: wih|whh|hw|xaug|b2col
    owih, owhh, ohw, oxa = 0, 3 * h, 6 * h, 6 * h + 8
    ob2 = oxa + t_steps
    nbtot = ob2 + 2
    blob_d = nc.dram_tensor("blob", [h + 1, nbtot], F16, kind="ExternalInput")
    out_d = nc.dram_tensor("out", [8, 1], F32, kind="ExternalOutput")

    with tile.TileContext(nc) as tc:
        with (
            tc.tile_pool(name="const", bufs=1) as const,
            tc.tile_pool(name="sm", bufs=6) as sm,
            tc.tile_pool(name="psa", bufs=3, space="PSUM") as psa,
            tc.tile_pool(name="psb", bufs=1, space="PSUM") as psb,
        ):
            # dummy activation: hoists the (serial) activation-table load to
            # kernel start, off the gi critical path
            dum = sm.tile([1, 1], F32, tag="dum")
            nc.vector.memset(dum[:], 0.0)
            nc.scalar.activation(dum[:], dum[:], AF.Sigmoid)

            bt = const.tile([h + 1, nbtot], F16, tag="bt")
            nc.sync.dma_start(bt[:], blob_d[:])
            wih = bt[:, owih:owih + 3 * h]
            whh = bt[:, owhh:owhh + 3 * h]
            hw = bt[:, ohw:ohw + 8]
            xa = bt[:, oxa:oxa + t_steps]
            b2c32 = bt[:, ob2:ob2 + 2].bitcast(F32)

            haug = const.tile([h + 1, 1], F16, tag="haug")
            nc.vector.memset(haug[0:h, :], 0.0)
            nc.vector.memset(haug[h:h + 1, :], 1.0)

            # phase 1 emits raw pre-bias embeddings; apply +b2 and relu here
            # (the augmented ones-row has b2=0 and is relu-invariant; b2 is
            # packed as fp32 inside the fp16 blob and bitcast on read)
            xar = const.tile([h + 1, t_steps], F16, tag="xar")
            nc.vector.tensor_scalar(xar[:], xa, b2c32, 0.0, OP.add, OP.max)

            psg = psa.tile([h, 3 * t_steps], F32, tag="psg")
            for j in range(3):
                nc.tensor.matmul(psg[:, j * t_steps:(j + 1) * t_steps],
                                 wih[:, j * h:(j + 1) * h], xar[:],
                                 start=True, stop=True)
            gi_all = const.tile([h, 3 * t_steps], F16, tag="giall")
            nc.vector.tensor_copy(gi_all[:], psg[:])
            gir = gi_all[:, 0:t_steps]
            giz = gi_all[:, t_steps:2 * t_steps]
            gin = gi_all[:, 2 * t_steps:3 * t_steps]

            for t in range(t_steps):
                psr = psb.tile([h, 1], F32, tag="psr")
                nc.tensor.matmul(psr[:], whh[:, 0:h], haug[:],
                                 start=True, stop=True)
                psz = psb.tile([h, 1], F32, tag="psz")
                nc.tensor.matmul(psz[:], whh[:, h:2 * h], haug[:],
                                 start=True, stop=True)
                psn = psb.tile([h, 1], F32, tag="psn")
                nc.tensor.matmul(psn[:], whh[:, 2 * h:3 * h], haug[:],
                                 start=True, stop=True)
                r = sm.tile([h, 1], F32, tag="r")
                nc.scalar.activation(r[:], psr[:], AF.Sigmoid,
                                     bias=gir[:, t:t + 1])
                z = sm.tile([h, 1], F32, tag="z")
                nc.scalar.activation(z[:], psz[:], AF.Sigmoid,
                                     bias=giz[:, t:t + 1])
                n_t = sm.tile([h, 1], F16, tag="nt")
                nc.scalar.activation(n_t[:], psn[:], AF.Tanh,
                                     bias=gin[:, t:t + 1], scale=r[:])
                hm = sm.tile([h, 1], F16, tag="hm")
                nc.vector.tensor_sub(hm[:], haug[0:h, :], n_t[:])
                nc.vector.scalar_tensor_tensor(haug[0:h, :], hm[:], z[:],
                                               n_t[:], OP.mult, OP.add)

            ps_o = psb.tile([8, 1], F32, tag="pso")
            nc.tensor.matmul(ps_o[:], hw, haug[:], start=True, stop=True)
            o = sm.tile([8, 1], F32, tag="o")
            nc.scalar.activation(o[:], ps_o[:], AF.Sigmoid)
            nc.sync.dma_start(out_d[:], o[:])
    nc.compile()
    return nc


_P1_CACHE = {}
_P2_CACHE = {}

# Dev/profiling knobs (test.py pokes these; harness leaves defaults).
TRACE = False
LAST_RES = {}


def _get_phase1(dims):
    key = tuple(dims)
    if key not in _P1_CACHE:
        nc = bacc.Bacc("TRN2", target_bir_lowering=False, debug=False,
                       num_devices=T)
        _P1_CACHE[key] = build_phase1(nc, dims)
    return _P1_CACHE[key]


def _get_phase2():
    key = (T, H)
    if key not in _P2_CACHE:
        nc = bacc.Bacc("TRN2", target_bir_lowering=False, debug=False,
                       num_devices=1)
        _P2_CACHE[key] = build_phase2(nc, T, H)
    return _P2_CACHE[key]


def _p2_blob(seq, Wih, Whh, bih, bhh, headW, headb, b2):
    h, t_steps = H, T
    owih, owhh, ohw, oxa = 0, 3 * h, 6 * h, 6 * h + 8
    ob2 = oxa + t_steps
    blob = np.zeros((h + 1, ob2 + 2), np.float16)
    blob[0:h, ob2:ob2 + 2] = \
        np.asarray(b2, np.float32).view(np.float16).reshape(h, 2)
    blob[0:h, owih:owih + 3 * h] = np.asarray(Wih, np.float16).T
    blob[h, owih:owih + 3 * h] = np.asarray(bih, np.float16)
    blob[0:h, owhh:owhh + 3 * h] = np.asarray(Whh, np.float16).T
    blob[h, owhh:owhh + 3 * h] = np.asarray(bhh, np.float16)
    blob[0:h, ohw:ohw + 8] = np.asarray(headW, np.float16).T
    blob[h, ohw:ohw + 8] = np.asarray(headb, np.float16)
    blob[0:h, oxa:oxa + t_steps] = np.asarray(seq, np.float16).T
    blob[h, oxa:oxa + t_steps] = 1.0
    return blob


def kernel(x, edge_index, edge_weight, W1, b1, W2, b2, Wih, Whh, bih, bhh,
           headW, headb):
    x = np.asarray(x, np.float32)
    edge_index = np.asarray(edge_index)
    edge_weight = np.asarray(edge_weight, np.float32)

    infos = [_analyze(np.asarray(edge_index[t, 0]),
                      np.asarray(edge_index[t, 1]), edge_weight[t])
             for t in range(T)]
    dims = _dims_from(infos)
    nc1 = _get_phase1(dims)

    in_maps = [_fill_blobs(infos[t], x[t], W1, W2, b1, b2, dims)
               for t in range(T)]
    res1 = bass_utils.run_bass_kernel_spmd(nc1, in_maps,
                                           core_ids=list(range(T)),
                                           trace=TRACE)
    LAST_RES["p1"] = res1
    seq = np.stack([np.asarray(res1.results[t]["india"]).reshape(H)
                    for t in range(T)])

    nc2 = _get_phase2()
    in2 = [{"blob": _p2_blob(seq, Wih, Whh, bih, bhh, headW, headb, b2)}]
    res2 = bass_utils.run_bass_kernel_spmd(nc2, in2, core_ids=[0],
                                           trace=TRACE)
    LAST_RES["p2"] = res2
    return np.asarray(res2.results[0]["out"]).reshape(8).astype(np.float32)


# revision 27
# speedup vs baseline: 38.5596x; 1.0040x over previous
"""Trainium2 Bass kernel for nn_SanctionImpactGNN.

Temporal GNN: per timestep t (T=8) a 2-layer GCN over a 20000-node /
320000-edge graph; node-0 ("india") embeddings over time feed a tiny GRU +
sigmoid heads -> [8] output.

Key observation
---------------
The reference returns only h2[india] per graph.  That value depends solely on
node 0's 2-hop in-neighborhood:

  * D  = {0} u in-neighbors(0)           (~15-20 nodes)   -- layer-1 outputs
  * A  = D u in-neighbors(D)             (~250-350 nodes) -- layer-1 sources
  * layer-1 edges: all edges with dst in D (~300)
  * layer-2 edges: all edges with dst = 0 (~15-20)
  * degrees (for the symmetric GCN norm) of every node in A, which need the
    full in-edge weight lists of those nodes (~5000 edge weights).

Everything else in the graph is dead code w.r.t. the output, so the kernel
computes exactly this subgraph.  The host does *index* work only (masking,
packing, permutation, dtype packing); every floating-point operation stays on
device.

Per-core (one graph snapshot per NeuronCore, data-parallel over T):
  * deg_A = 1 + rowsum(ew slots)  -> dis_A = 1/sqrt(deg_A)   [partition axis]
  * deg_D via ones-matmul on a transposed slot pack -> dis as a row, expanded
    to disX[h,d] by a rank-1 matmul (keeps every dis scale off the critical
    path and avoids any PE transpose of activations)
  * g1 = dis_A * (x[A] @ W1)
  * h1^T = relu((g1^T-contracted M1 matmul) * disX + b1)   [M1 layered dense
    weighted adjacency; duplicate edges/self-loop collisions get their own
    layer so the host never sums weights]
  * g2 = h1 @ W2;  h2[0] = relu(dis_0 * (M3s^T @ g2) + b2)  [M3s rows are
    dis_D-scaled on device]
All matmuls run in fp16 (inputs quantized host-side; PSUM accumulates fp32).

Phase 2 (single core): 8-step GRU + sigmoid heads, biases folded via
augmented-ones rows, one fp16 blob load, gi for all steps precomputed, gate
math fused into Act ops (sigmoid/tanh with AP scale/bias).
"""

import numpy as np

import concourse.bacc as bacc
import concourse.mybir as mybir
import concourse.tile as tile
from concourse import bass_utils

F32 = mybir.dt.float32
F16 = mybir.dt.float16
AF = mybir.ActivationFunctionType
OP = mybir.AluOpType
AX = mybir.AxisListType

# Problem constants (hardcoded per contest contract).
T, N, E, F, H = 8, 20000, 320000, 128, 64
P = 128
INDIA = 0
CAP_D = 32  # max |{0} u in-neighbors(0)| supported (observed ~20)


def _analyze(src, dst, w):
    """Pure-index extraction of node 0's 2-hop in-neighborhood."""
    m0 = dst == INDIA
    s1 = np.unique(src[m0])
    D = np.concatenate([[INDIA], s1[s1 != INDIA]]).astype(np.int64)
    nD = len(D)
    assert nD <= CAP_D, f"|D|={nD} exceeds CAP_D={CAP_D}"
    mDe = np.isin(dst, D)
    extra = np.setdiff1d(np.unique(src[mDe]), D)
    A = np.concatenate([D, extra.astype(np.int64)])
    nA = len(A)
    pos = np.full(N, -1, np.int64)
    pos[A] = np.arange(nA)

    # per-A-node in-edge weight slots (partition-axis degree layout)
    mA = np.isin(dst, A)
    rdeg = pos[dst[mA]]
    o = np.argsort(rdeg, kind="stable")
    rdeg = rdeg[o]
    kdeg = np.arange(len(rdeg)) - np.searchsorted(rdeg, rdeg, "left")
    vdeg = np.asarray(w[mA], np.float32)[o]

    # layer-1 weighted adjacency entries (incl. unit self-loops), layered so
    # coincident (src,dst) cells never need host-side summation
    sM = np.concatenate([pos[src[mDe]], np.arange(nD)])
    dM = np.concatenate([pos[dst[mDe]], np.arange(nD)])
    vM = np.concatenate([np.asarray(w[mDe], np.float32),
                         np.ones(nD, np.float32)])
    assert (sM >= 0).all() and (dM >= 0).all() and (dM < nD).all()
    key = sM * CAP_D + dM
    o1 = np.argsort(key, kind="stable")
    ks = key[o1]
    lM = np.arange(len(ks)) - np.searchsorted(ks, ks, "left")

    # layer-2 entries: edges into node 0 (+ its self-loop)
    s3 = np.concatenate([pos[src[m0]], [0]])
    v3 = np.concatenate([np.asarray(w[m0], np.float32),
                         np.ones(1, np.float32)])
    assert (s3 >= 0).all() and (s3 < nD).all()
    o3 = np.argsort(s3, kind="stable")
    s3 = s3[o3]
    l3 = np.arange(len(s3)) - np.searchsorted(s3, s3, "left")

    return dict(A=A, nA=nA, nD=nD,
                deg_r=rdeg, deg_k=kdeg, deg_v=vdeg,
                m1_s=sM[o1], m1_d=dM[o1], m1_l=lM, m1_v=vM[o1],
                m3_s=s3, m3_l=l3, m3_v=v3[o3])


def _dims_from(infos):
    G = max(1, -(-max(i["nA"] for i in infos) // P))
    capdeg = max(8, int(max(i["deg_k"].max() + 1 if len(i["deg_k"]) else 1
                            for i in infos)))
    capdeg = (capdeg + 3) // 4 * 4
    L1 = int(max(i["m1_l"].max() + 1 for i in infos))
    L3 = int(max(i["m3_l"].max() + 1 for i in infos))
    return G, capdeg, L1, L3


def _blob1_offsets(dims):
    G, capdeg, L1, L3 = dims
    o = {}
    c = 0
    for name, width in (("ewdeg", G * capdeg), ("w1", H),
                        ("ewD", max(capdeg, CAP_D)), ("ones_c", 1),
                        ("ones_r", H), ("m1", L1 * G * CAP_D),
                        ("m3", L3 * CAP_D), ("w2", H), ("b1r", H)):
        o[name] = c
        c += width
    return o, c


def _fill_blobs(info, x_t, W1, W2, b1, b2, dims):
    G, capdeg, L1, L3 = dims
    nA, nD = info["nA"], info["nD"]
    o1, nb1 = _blob1_offsets(dims)
    blob1 = np.zeros((P, nb1), np.float16)
    blob2 = np.zeros((P, G * P), np.float16)
    blob2[:, 0:nA] = np.asarray(x_t, np.float32)[info["A"]].T.astype(np.float16)

    ewdeg = np.zeros((P, G, capdeg), np.float16)
    ewdeg[info["deg_r"] % P, info["deg_r"] // P, info["deg_k"]] = info["deg_v"]
    blob1[:, o1["ewdeg"]:o1["ewdeg"] + G * capdeg] = ewdeg.reshape(P, -1)
    blob1[:, o1["w1"]:o1["w1"] + H] = np.asarray(W1, np.float16)
    # transposed in-edge slots for D nodes only (slot on partition, node on
    # free) -> degrees of D as a row via ones-matmul
    mD = info["deg_r"] < nD
    blob1[info["deg_k"][mD], o1["ewD"] + info["deg_r"][mD]] = \
        info["deg_v"][mD].astype(np.float16)
    blob1[0:capdeg, o1["ones_c"]] = 1.0
    blob1[0, o1["ones_r"]:o1["ones_r"] + H] = 1.0

    m1 = np.zeros((P, L1, G, CAP_D), np.float16)
    m1[info["m1_s"] % P, info["m1_l"], info["m1_s"] // P,
       info["m1_d"]] = info["m1_v"]
    blob1[:, o1["m1"]:o1["m1"] + L1 * G * CAP_D] = m1.reshape(P, -1)
    m3 = np.zeros((L3, CAP_D), np.float16)
    m3[info["m3_l"], info["m3_s"]] = info["m3_v"]
    blob1[0, o1["m3"]:o1["m3"] + L3 * CAP_D] = m3.reshape(-1)
    blob1[0:H, o1["w2"]:o1["w2"] + H] = np.asarray(W2, np.float16)
    blob1[0, o1["b1r"]:o1["b1r"] + H] = np.asarray(b1, np.float16)
    return {"blob1": blob1, "blob2": blob2}


def build_phase1(nc, dims):
    G, capdeg, L1, L3 = dims
    o1, nb1 = _blob1_offsets(dims)

    b1_d = nc.dram_tensor("blob1", [P, nb1], F16, kind="ExternalInput")
    b2_d = nc.dram_tensor("blob2", [P, G * P], F16, kind="ExternalInput")
    india_d = nc.dram_tensor("india", [H, 1], F32, kind="ExternalOutput")

    with tile.TileContext(nc) as tc:
        with (
            tc.tile_pool(name="const", bufs=1) as const,
            tc.tile_pool(name="sm", bufs=8) as sm,
            tc.tile_pool(name="psa", bufs=3, space="PSUM") as psa,
            tc.tile_pool(name="psb", bufs=1, space="PSUM") as psb,
        ):
            b1t = const.tile([P, nb1], F16, tag="b1t")
            b2t = const.tile([P, G * P], F16, tag="b2t")
            nc.sync.dma_start(b1t[:], b1_d[:])
            nc.sync.dma_start(b2t[:], b2_d[:])

            ewd3 = b1t[:, o1["ewdeg"]:o1["ewdeg"] + G * capdeg].rearrange(
                "p (g c) -> p g c", c=capdeg)
            xTv = b2t[:, :]
            w1v = b1t[:, o1["w1"]:o1["w1"] + H]
            ewD = b1t[0:capdeg, o1["ewD"]:o1["ewD"] + capdeg]
            ones_c = b1t[0:capdeg, o1["ones_c"]:o1["ones_c"] + 1]
            ones_r = b1t[0:1, o1["ones_r"]:o1["ones_r"] + H]
            m1v = b1t[:, o1["m1"]:o1["m1"] + L1 * G * CAP_D]
            m3v = b1t[0:1, o1["m3"]:o1["m3"] + L3 * CAP_D]
            w2v = b1t[0:H, o1["w2"]:o1["w2"] + H]
            b1r = b1t[0:1, o1["b1r"]:o1["b1r"] + H]

            # dis over all A nodes (partition layout) for the g1 scale
            deg = sm.tile([P, G], F32, tag="deg")
            dis = sm.tile([P, G], F32, tag="dis")
            for g in range(G):
                nc.vector.reduce_sum(deg[:, g:g + 1], ewd3[:, g:g + 1, :],
                                     axis=AX.X)
                nc.scalar.activation(deg[:, g:g + 1], deg[:, g:g + 1],
                                     AF.Sqrt, bias=1.0)
                nc.vector.reciprocal(dis[:, g:g + 1], deg[:, g:g + 1])

            # dis over D as a row -> disX[h, d] = dis_d (rank-1 matmul)
            psdr = psb.tile([1, CAP_D], F32, tag="psdr")
            nc.tensor.matmul(psdr[:], ones_c, ewD[:, 0:CAP_D],
                             start=True, stop=True)
            sqr = sm.tile([1, CAP_D], F32, tag="sqr")
            nc.scalar.activation(sqr[:], psdr[:], AF.Sqrt, bias=1.0)
            dr = sm.tile([1, CAP_D], F32, tag="dr")
            nc.vector.reciprocal(dr[:], sqr[:])

            # g1 = dis_A * (x[A] @ W1)
            g1 = const.tile([P, G * H], F16, tag="g1")
            for g in range(G):
                psg = psa.tile([P, H], F32, tag="psg")
                nc.tensor.matmul(psg[:], xTv[:, g * P:(g + 1) * P], w1v,
                                 start=True, stop=True)
                if g == 2:
                    nc.scalar.mul(g1[:, g * H:(g + 1) * H], psg[:],
                                  dis[:, g:g + 1])
                else:
                    nc.vector.tensor_scalar_mul(g1[:, g * H:(g + 1) * H],
                                                psg[:], dis[:, g:g + 1])


            # layer-1 aggregation, transposed: ps1t[h, d] = sum_s g1[s,h]M1[s,d]
            # (off critical path) layer-2 row: QX[l,d] = dis0 * dis_d^2 * M3[l,d], expanded over
            # h by a rank-1 matmul (Pool engine; off the critical path)
            drsq = sm.tile([1, CAP_D], F16, tag="drsq")
            nc.gpsimd.tensor_tensor(drsq[:], dr[:], dr[:], op=OP.mult)
            m3dr = sm.tile([1, L3 * CAP_D], F16, tag="m3dr")
            nc.vector.scalar_tensor_tensor(
                m3dr[:], m3v, dr[0:1, 0:1],
                drsq[:].unsqueeze(1).broadcast_to((1, L3, CAP_D)),
                OP.mult, OP.mult)
            psm3 = psb.tile([H, L3 * CAP_D], F32, tag="psm3")
            nc.tensor.matmul(psm3[:], ones_r, m3dr[:], start=True, stop=True)
            qx = sm.tile([H, L3 * CAP_D], F16, tag="qx")
            nc.vector.tensor_copy(qx[:], psm3[:])
            sqrh = sm.tile([1, CAP_D], F16, tag="sqrh")
            nc.scalar.copy(sqrh[:], sqr[:])

            # ps1t[h,d] = sum_s g1[s,h] M1[s,d] + b1[h] sqrt(deg_d+1); with
            # that bias row folded in, relu commutes past the positive scales:
            # u2 = max(ps1t, 0) * QX in a single fused DVE op
            ps1t = psb.tile([H, CAP_D], F32, tag="ps1t")
            nc.tensor.matmul(ps1t[:], b1r, sqrh[:], start=True, stop=False)
            k, nmm = 0, L1 * G
            for l in range(L1):
                for g in range(G):
                    nc.tensor.matmul(ps1t[:], g1[:, g * H:(g + 1) * H],
                                     m1v[:, (l * G + g) * CAP_D:
                                         (l * G + g + 1) * CAP_D],
                                     start=False, stop=(k == nmm - 1))
                    k += 1

            # layer 2 collapsed: india[k] = sum_h W2[h,k] * sum_{l,d}
            #   max(ps1t[h,d],0) * QX[h,(l,d)]  (+b2, relu applied in phase 2)
            u2 = sm.tile([H, L3 * CAP_D], F16, tag="u2")
            nc.vector.scalar_tensor_tensor(
                u2[:], ps1t[:].unsqueeze(1).broadcast_to((H, L3, CAP_D)), 0.0,
                qx[:].rearrange("p (l d) -> p l d", d=CAP_D), OP.max, OP.mult)
            ps4 = psb.tile([H, L3 * CAP_D], F32, tag="ps4")
            nc.tensor.matmul(ps4[:], w2v, u2[:], start=True, stop=True)
            t2c = sm.tile([H, 1], F32, tag="t2c")
            nc.vector.reduce_sum(t2c[:], ps4[:], axis=AX.X)
            nc.sync.dma_start(india_d[:], t2c[:])
    nc.compile()
    return nc


def build_phase2(nc, t_steps, h):
    # column layout: wih|whh|hw|xaug|b2col
    owih, owhh, ohw, oxa = 0, 3 * h, 6 * h, 6 * h + 8
    ob2 = oxa + t_steps
    nbtot = ob2 + 2
    blob_d = nc.dram_tensor("blob", [h + 1, nbtot], F16, kind="ExternalInput")
    out_d = nc.dram_tensor("out", [8, 1], F32, kind="ExternalOutput")

    with tile.TileContext(nc) as tc:
        with (
            tc.tile_pool(name="const", bufs=1) as const,
            tc.tile_pool(name="sm", bufs=6) as sm,
            tc.tile_pool(name="psa", bufs=3, space="PSUM") as psa,
            tc.tile_pool(name="psb", bufs=1, space="PSUM") as psb,
        ):
            # dummy activation: hoists the (serial) activation-table load to
            # kernel start, off the gi critical path
            dum = sm.tile([1, 1], F32, tag="dum")
            nc.vector.memset(dum[:], 0.0)
            nc.scalar.activation(dum[:], dum[:], AF.Sigmoid)

            bt = const.tile([h + 1, nbtot], F16, tag="bt")
            nc.sync.dma_start(bt[:], blob_d[:])
            wih = bt[:, owih:owih + 3 * h]
            whh = bt[:, owhh:owhh + 3 * h]
            hw = bt[:, ohw:ohw + 8]
            xa = bt[:, oxa:oxa + t_steps]
            b2c32 = bt[:, ob2:ob2 + 2].bitcast(F32)

            haug = const.tile([h + 1, 1], F16, tag="haug")
            nc.vector.memset(haug[0:h, :], 0.0)
            nc.vector.memset(haug[h:h + 1, :], 1.0)

            # phase 1 emits raw pre-bias embeddings; apply +b2 and relu here
            # (the augmented ones-row has b2=0 and is relu-invariant; b2 is
            # packed as fp32 inside the fp16 blob and bitcast on read)
            xar = const.tile([h + 1, t_steps], F16, tag="xar")
            nc.vector.tensor_scalar(xar[:], xa, b2c32, 0.0, OP.add, OP.max)

            psg = psa.tile([h, 3 * t_steps], F32, tag="psg")
            for j in range(3):
                nc.tensor.matmul(psg[:, j * t_steps:(j + 1) * t_steps],
                                 wih[:, j * h:(j + 1) * h], xar[:],
                                 start=True, stop=True)
            gi_all = const.tile([h, 3 * t_steps], F16, tag="giall")
            nc.vector.tensor_copy(gi_all[:], psg[:])
            gir = gi_all[:, 0:t_steps]
            giz = gi_all[:, t_steps:2 * t_steps]
            gin = gi_all[:, 2 * t_steps:3 * t_steps]

            for t in range(t_steps):
                psr = psb.tile([h, 1], F32, tag="psr")
                nc.tensor.matmul(psr[:], whh[:, 0:h], haug[:],
                                 start=True, stop=True)
                psz = psb.tile([h, 1], F32, tag="psz")
                nc.tensor.matmul(psz[:], whh[:, h:2 * h], haug[:],
                                 start=True, stop=True)
                psn = psb.tile([h, 1], F32, tag="psn")
                nc.tensor.matmul(psn[:], whh[:, 2 * h:3 * h], haug[:],
                                 start=True, stop=True)
                r = sm.tile([h, 1], F32, tag="r")
                nc.scalar.activation(r[:], psr[:], AF.Sigmoid,
                                     bias=gir[:, t:t + 1])
                z = sm.tile([h, 1], F32, tag="z")
                nc.scalar.activation(z[:], psz[:], AF.Sigmoid,
                                     bias=giz[:, t:t + 1])
                n_t = sm.tile([h, 1], F16, tag="nt")
                nc.scalar.activation(n_t[:], psn[:], AF.Tanh,
                                     bias=gin[:, t:t + 1], scale=r[:])
                hm = sm.tile([h, 1], F16, tag="hm")
                nc.vector.tensor_sub(hm[:], haug[0:h, :], n_t[:])
                nc.vector.scalar_tensor_tensor(haug[0:h, :], hm[:], z[:],
                                               n_t[:], OP.mult, OP.add)

            ps_o = psb.tile([8, 1], F32, tag="pso")
            nc.tensor.matmul(ps_o[:], hw, haug[:], start=True, stop=True)
            o = sm.tile([8, 1], F32, tag="o")
            nc.scalar.activation(o[:], ps_o[:], AF.Sigmoid)
            nc.sync.dma_start(out_d[:], o[:])
    nc.compile()
    return nc


_P1_CACHE = {}
_P2_CACHE = {}

# Dev/profiling knobs (test.py pokes these; harness leaves defaults).
TRACE = False
LAST_RES = {}


def _get_phase1(dims):
    key = tuple(dims)
    if key not in _P1_CACHE:
        nc = bacc.Bacc("TRN2", target_bir_lowering=False, debug=False,
                       num_devices=T)
        _P1_CACHE[key] = build_phase1(nc, dims)
    return _P1_CACHE[key]


def _get_phase2():
    key = (T, H)
    if key not in _P2_CACHE:
        nc = bacc.Bacc("TRN2", target_bir_lowering=False, debug=False,
                       num_devices=1)
        _P2_CACHE[key] = build_phase2(nc, T, H)
    return _P2_CACHE[key]


def _p2_blob(seq, Wih, Whh, bih, bhh, headW, headb, b2):
    h, t_steps = H, T
    owih, owhh, ohw, oxa = 0, 3 * h, 6 * h, 6 * h + 8
    ob2 = oxa + t_steps
    blob = np.zeros((h + 1, ob2 + 2), np.float16)
    blob[0:h, ob2:ob2 + 2] = \
        np.asarray(b2, np.float32).view(np.float16).reshape(h, 2)
    blob[0:h, owih:owih + 3 * h] = np.asarray(Wih, np.float16).T
    blob[h, owih:owih + 3 * h] = np.asarray(bih, np.float16)
    blob[0:h, owhh:owhh + 3 * h] = np.asarray(Whh, np.float16).T
    blob[h, owhh:owhh + 3 * h] = np.asarray(bhh, np.float16)
    blob[0:h, ohw:ohw + 8] = np.asarray(headW, np.float16).T
    blob[h, ohw:ohw + 8] = np.asarray(headb, np.float16)
    blob[0:h, oxa:oxa + t_steps] = np.asarray(seq, np.float16).T
    blob[h, oxa:oxa + t_steps] = 1.0
    return blob


def kernel(x, edge_index, edge_weight, W1, b1, W2, b2, Wih, Whh, bih, bhh,
           headW, headb):
    x = np.asarray(x, np.float32)
    edge_index = np.asarray(edge_index)
    edge_weight = np.asarray(edge_weight, np.float32)

    infos = [_analyze(np.asarray(edge_index[t, 0]),
                      np.asarray(edge_index[t, 1]), edge_weight[t])
             for t in range(T)]
    dims = _dims_from(infos)
    nc1 = _get_phase1(dims)

    in_maps = [_fill_blobs(infos[t], x[t], W1, W2, b1, b2, dims)
               for t in range(T)]
    res1 = bass_utils.run_bass_kernel_spmd(nc1, in_maps,
                                           core_ids=list(range(T)),
                                           trace=TRACE)
    LAST_RES["p1"] = res1
    seq = np.stack([np.asarray(res1.results[t]["india"]).reshape(H)
                    for t in range(T)])

    nc2 = _get_phase2()
    in2 = [{"blob": _p2_blob(seq, Wih, Whh, bih, bhh, headW, headb, b2)}]
    res2 = bass_utils.run_bass_kernel_spmd(nc2, in2, core_ids=[0],
                                           trace=TRACE)
    LAST_RES["p2"] = res2
    return np.asarray(res2.results[0]["out"]).reshape(8).astype(np.float32)
